# revision 1
# baseline (speedup 1.0000x reference)
"""ContentAwareMambaFilter Trainium2 kernel.

Data-parallel over batch: 8 NeuronCores, one batch row each. Takes full
(unsharded) inputs, returns the full output; per-core slicing happens in
kernel(). The Bass program is built and compiled once, then cached.

Per-core pipeline (everything [features-on-partitions, time-on-free]):
  A: transpose x via PE, FiLM MLP on PE/ACT, x_mod in SBUF
  B: in_proj on PE, depthwise causal conv on DVE, silu on ACT;
     xc and silu(z) spilled to DRAM scratch
  C: x_proj on PE -> dt_in [48,L] SBUF, B/C rows -> DRAM scratch
  D: per 512-step block x 12 channel-chunks: dt = softplus via Ln(1+Exp),
     decay a = Exp(A[:,n]*dt) per state (ACT, per-partition scale),
     u = dt*xc*B (DVE, step-0 broadcast AP), hardware scan
     (tensor_tensor_scan) over 8-state sections with carry fix-up,
     y = sum_n C*h (strided reduce), gate with silu(z), out_proj on PE
     accumulating [t,dim] in PSUM, then residual + LayerNorm, store.
"""

import numpy as np

B = 8
L = 2048
DIM = 768
DSTATE = 16
DCONV = 4
DINNER = 1536
DTRANK = 48

NCH = DINNER // 128          # 12 channel chunks
CCH = DIM // 128             # 6 dim chunks
TB = 512                     # scan time block
NBLK = L // TB
NTT = L // 512               # matmul t tiles
NGRP = 2                     # state groups per scan pass
GS = DSTATE // NGRP          # 8 states per group
EPS = 1e-5

_CACHE = {}


def _build():
    from contextlib import ExitStack
    import concourse.bacc as bacc
    import concourse.tile as tile
    import concourse.mybir as mybir
    from concourse.masks import make_identity

    f32 = mybir.dt.float32
    bf16 = mybir.dt.bfloat16
    AF = mybir.ActivationFunctionType
    OP = mybir.AluOpType
    AX = mybir.AxisListType

    nc = bacc.Bacc("TRN2", target_bir_lowering=False, debug=False)

    x_d = nc.dram_tensor("x", [L, DIM], f32, kind="ExternalInput").ap()
    sal_d = nc.dram_tensor("sal", [L, 1], f32, kind="ExternalInput").ap()
    spw1_d = nc.dram_tensor("sp_w1", [1, DIM // 4], f32, kind="ExternalInput").ap()
    spb1_d = nc.dram_tensor("sp_b1", [DIM // 4], f32, kind="ExternalInput").ap()
    spw2_d = nc.dram_tensor("sp_w2", [DIM // 4, 2 * DIM], f32, kind="ExternalInput").ap()
    spb2_d = nc.dram_tensor("sp_b2", [2 * DIM], f32, kind="ExternalInput").ap()
    win_d = nc.dram_tensor("in_proj_w", [DIM, 2 * DINNER], f32, kind="ExternalInput").ap()
    wcv_d = nc.dram_tensor("conv_w", [DINNER, DCONV], f32, kind="ExternalInput").ap()
    cvb_d = nc.dram_tensor("conv_b", [DINNER], f32, kind="ExternalInput").ap()
    wxp_d = nc.dram_tensor("x_proj_w", [DINNER, DTRANK + 2 * DSTATE], f32, kind="ExternalInput").ap()
    wdt_d = nc.dram_tensor("dt_proj_w", [DTRANK, DINNER], f32, kind="ExternalInput").ap()
    dtb_d = nc.dram_tensor("dt_proj_b", [DINNER], f32, kind="ExternalInput").ap()
    alog_d = nc.dram_tensor("A_log", [DINNER, DSTATE], f32, kind="ExternalInput").ap()
    dD_d = nc.dram_tensor("D", [DINNER], f32, kind="ExternalInput").ap()
    wout_d = nc.dram_tensor("out_proj_w", [DINNER, DIM], f32, kind="ExternalInput").ap()
    lng_d = nc.dram_tensor("ln_g", [DIM], f32, kind="ExternalInput").ap()
    lnb_d = nc.dram_tensor("ln_b", [DIM], f32, kind="ExternalInput").ap()
    out_d = nc.dram_tensor("out", [L, DIM], f32, kind="ExternalOutput").ap()

    xc_d = nc.dram_tensor("xc_scr", [NCH, 128, L], f32).ap()
    zs_d = nc.dram_tensor("zs_scr", [NCH, 128, L], f32).ap()
    bc_d = nc.dram_tensor("bc_scr", [2, DSTATE, L], f32).ap()

    with tile.TileContext(nc) as tc, ExitStack() as ctx:
        # ---------- long-lived constants ----------
        consts = ctx.enter_context(tc.tile_pool(name="consts", bufs=1))

        A_t = []
        for i in range(NCH):
            al = consts.tile([128, DSTATE], f32, tag=f"alog{i}")
            nc.sync.dma_start(al[:], alog_d[i * 128:(i + 1) * 128, :])
            at = consts.tile([128, DSTATE], f32, tag=f"at{i}")
            nc.scalar.activation(at[:], al[:], AF.Exp)
            nc.vector.tensor_scalar_mul(at[:], at[:], -1.0)
            A_t.append(at)

        def col_per_chunk(src_vec, name):
            t = consts.tile([128, NCH], f32, tag=name)
            nc.sync.dma_start(
                t[:], src_vec.rearrange("(i p) -> i p", p=128).transpose([1, 0]))
            return t

        dtpb = col_per_chunk(dtb_d, "dtpb")
        dDc = col_per_chunk(dD_d, "dDc")
        lngb = consts.tile([128, DIM], f32, tag="lngb")
        nc.sync.dma_start(lngb[:], lng_d.partition_broadcast(128))
        lnbb = consts.tile([128, DIM], f32, tag="lnbb")
        nc.sync.dma_start(lnbb[:], lnb_d.partition_broadcast(128))
        dtw = []
        for i in range(NCH):
            t = consts.tile([DTRANK, 128], f32, tag=f"dtw{i}")
            nc.sync.dma_start(t[:], wdt_d[:, i * 128:(i + 1) * 128])
            dtw.append(t)
        dtin_sb = consts.tile([DTRANK, L], f32, tag="dtin")
        epsc = consts.tile([128, 1], f32, tag="epsc")
        nc.vector.memset(epsc[:], EPS)
        cys = [consts.tile([128, DSTATE], f32, tag=f"cy{i}", name=f"cy{i}") for i in range(NCH)]

        # ---------- phases A + B (x_mod lives across both) ----------
        with tc.tile_pool(name="xmod", bufs=1) as xmod_pool:
            xmod = [xmod_pool.tile([128, L], f32, tag=f"xm{cc}", name=f"xm{cc}") for cc in range(CCH)]

            with tc.tile_pool(name="pa", bufs=2) as pA, \
                 tc.tile_pool(name="pa_c", bufs=1) as pAc, \
                 tc.tile_pool(name="pa_ps", bufs=2, space="PSUM") as pA_ps:
                ident = pAc.tile([128, 128], f32, tag="ident")
                make_identity(nc, ident[:])
                ones96 = pAc.tile([1, 96], f32, tag="ones96")
                nc.vector.memset(ones96[:], 1.0)
                w1c = pAc.tile([96, 2], f32, tag="w1c")
                nc.sync.dma_start(
                    w1c[:], spw1_d.rearrange("o (g j) -> o g j", g=2).squeeze(0).transpose([1, 0]))
                b1c = pAc.tile([96, 2], f32, tag="b1c")
                nc.sync.dma_start(b1c[:], spb1_d.rearrange("(g j) -> g j", g=2).transpose([1, 0]))
                spb2c = pAc.tile([128, 12], f32, tag="spb2")
                nc.sync.dma_start(
                    spb2c[:], spb2_d.rearrange("(i p) -> i p", p=128).transpose([1, 0]))
                w2c = []
                for kc in range(2):
                    row = []
                    for m in range(12):
                        t = pAc.tile([96, 128], f32, tag=f"w2c{kc}_{m}")
                        nc.sync.dma_start(
                            t[:], spw2_d[kc * 96:(kc + 1) * 96, m * 128:(m + 1) * 128])
                        row.append(t)
                    w2c.append(row)

                # saliency broadcast + FiLM hidden layer
                sal_sb = pAc.tile([1, L], f32, tag="salsb")
                nc.sync.dma_start(sal_sb[:], sal_d.transpose([1, 0]))
                h2 = [pAc.tile([96, L], f32, tag=f"h2_{kc}", name=f"h2_{kc}") for kc in range(2)]
                for kc in range(2):
                    for tt in range(NTT):
                        ps = pA_ps.tile([96, 512], f32, tag="salps")
                        nc.tensor.matmul(ps[:], ones96[:],
                                         sal_sb[:, tt * 512:(tt + 1) * 512],
                                         start=True, stop=True)
                        nc.scalar.activation(h2[kc][:, tt * 512:(tt + 1) * 512], ps[:],
                                             AF.Relu, scale=w1c[:, kc:kc + 1],
                                             bias=b1c[:, kc:kc + 1])

                # x transpose -> xmod tiles hold xT for now
                for cc in range(CCH):
                    for tcn in range(L // 128):
                        xt_in = pA.tile([128, 128], f32, tag="xtin")
                        nc.sync.dma_start(
                            xt_in[:], x_d[tcn * 128:(tcn + 1) * 128, cc * 128:(cc + 1) * 128])
                        ps = pA_ps.tile([128, 128], f32, tag="xtps")
                        nc.tensor.transpose(ps[:], xt_in[:], ident[:])
                        nc.scalar.copy(xmod[cc][:, tcn * 128:(tcn + 1) * 128], ps[:])

                # FiLM affine + modulation, per (cc, tt) tile
                for cc in range(CCH):
                    for tt in range(NTT):
                        sl = slice(tt * 512, (tt + 1) * 512)
                        psg = pA_ps.tile([128, 512], f32, tag="affg")
                        for kc in range(2):
                            nc.tensor.matmul(psg[:], w2c[kc][cc][:], h2[kc][:, sl],
                                             start=(kc == 0), stop=(kc == 1))
                        tg = pA.tile([128, 512], f32, tag="tg")
                        nc.scalar.activation(tg[:], psg[:], AF.Tanh,
                                             bias=spb2c[:, cc:cc + 1])
                        psb = pA_ps.tile([128, 512], f32, tag="affb")
                        for kc in range(2):
                            nc.tensor.matmul(psb[:], w2c[kc][cc + 6][:], h2[kc][:, sl],
                                             start=(kc == 0), stop=(kc == 1))
                        bt = pA.tile([128, 512], f32, tag="bt")
                        nc.scalar.activation(bt[:], psb[:], AF.Identity,
                                             bias=spb2c[:, cc + 6:cc + 7])
                        nc.vector.tensor_scalar_add(tg[:], tg[:], 1.0)
                        nc.vector.tensor_tensor(tg[:], xmod[cc][:, sl], tg[:], OP.mult)
                        nc.vector.tensor_tensor(xmod[cc][:, sl], tg[:], bt[:], OP.add)

            # ---------- phase B ----------
            with tc.tile_pool(name="pb", bufs=2) as pB, \
                 tc.tile_pool(name="pb_c", bufs=1) as pBc, \
                 tc.tile_pool(name="pb_w", bufs=3) as pB_w, \
                 tc.tile_pool(name="pb_ps", bufs=3, space="PSUM") as pB_ps:
                wcv = pBc.tile([128, NCH * DCONV], f32, tag="wcv")
                nc.sync.dma_start(
                    wcv[:], wcv_d.rearrange("(i p) k -> i p k", p=128).transpose([1, 0, 2]))
                cvb = pBc.tile([128, NCH], f32, tag="cvb")
                nc.sync.dma_start(
                    cvb[:], cvb_d.rearrange("(i p) -> i p", p=128).transpose([1, 0]))

                for m in range(24):
                    psl = [pB_ps.tile([128, 512], f32, tag=f"ipp{tt % 2}", name=f"ipp{m}_{tt}")
                           for tt in range(NTT)]
                    for cc in range(CCH):
                        wt = pB_w.tile([128, 128], f32, tag="wstage")
                        nc.sync.dma_start(
                            wt[:], win_d[cc * 128:(cc + 1) * 128, m * 128:(m + 1) * 128])
                        for tt in range(NTT):
                            nc.tensor.matmul(psl[tt][:], wt[:],
                                             xmod[cc][:, tt * 512:(tt + 1) * 512],
                                             start=(cc == 0), stop=(cc == CCH - 1))
                    if m >= 12:
                        i = m - 12
                        for tt in range(NTT):
                            zt = pB.tile([128, 512], f32, tag="ztile")
                            nc.scalar.activation(zt[:], psl[tt][:], AF.Silu)
                            nc.sync.dma_start(zs_d[i, :, tt * 512:(tt + 1) * 512], zt[:])
                    else:
                        i = m
                        xin = pB.tile([128, L + 3], f32, tag="xin")
                        nc.vector.memset(xin[:, 0:3], 0.0)
                        for tt in range(NTT):
                            nc.scalar.copy(xin[:, 3 + tt * 512:3 + (tt + 1) * 512],
                                           psl[tt][:])
                        acc = pB.tile([128, L], f32, tag="cacc")
                        acc2 = pB.tile([128, L], f32, tag="cacc2")
                        nc.vector.tensor_scalar_mul(
                            acc[:], xin[:, 0:L], wcv[:, i * DCONV:i * DCONV + 1])
                        nc.vector.scalar_tensor_tensor(
                            acc2[:], xin[:, 1:1 + L],
                            wcv[:, i * DCONV + 1:i * DCONV + 2], acc[:],
                            op0=OP.mult, op1=OP.add)
                        nc.vector.scalar_tensor_tensor(
                            acc[:], xin[:, 2:2 + L],
                            wcv[:, i * DCONV + 2:i * DCONV + 3], acc2[:],
                            op0=OP.mult, op1=OP.add)
                        nc.vector.scalar_tensor_tensor(
                            acc2[:], xin[:, 3:3 + L],
                            wcv[:, i * DCONV + 3:i * DCONV + 4], acc[:],
                            op0=OP.mult, op1=OP.add)
                        xct = pB.tile([128, L], f32, tag="xct")
                        nc.scalar.activation(xct[:], acc2[:], AF.Silu,
                                             bias=cvb[:, i:i + 1])
                        nc.sync.dma_start(xc_d[i], xct[:])

        # ---------- phase C ----------
        with tc.tile_pool(name="pc", bufs=2) as pC, \
             tc.tile_pool(name="pc_c", bufs=1) as pCc, \
             tc.tile_pool(name="pc_ps", bufs=1, space="PSUM") as pC_ps:
            # stationary padded to 112 cols: dt 0:48, B 64:80, C 96:112 so the
            # PSUM rows land on 32-aligned partition bases.
            xpw = []
            for i in range(NCH):
                t = pCc.tile([128, 112], f32, tag=f"xpw{i}")
                nc.vector.memset(t[:], 0.0)
                isl = slice(i * 128, (i + 1) * 128)
                nc.sync.dma_start(t[:, 0:DTRANK], wxp_d[isl, 0:DTRANK])
                nc.sync.dma_start(t[:, 64:80], wxp_d[isl, DTRANK:DTRANK + DSTATE])
                nc.sync.dma_start(t[:, 96:112], wxp_d[isl, DTRANK + DSTATE:])
                xpw.append(t)
            psd = [pC_ps.tile([112, 512], f32, tag=f"dtbc{tt}", name=f"dtbc{tt}")
                   for tt in range(NTT)]
            for i in range(NCH):
                xci = pC.tile([128, L], f32, tag="xcld")
                nc.sync.dma_start(xci[:], xc_d[i])
                for tt in range(NTT):
                    nc.tensor.matmul(psd[tt][:], xpw[i][:],
                                     xci[:, tt * 512:(tt + 1) * 512],
                                     start=(i == 0), stop=(i == NCH - 1))
            for tt in range(NTT):
                sl = slice(tt * 512, (tt + 1) * 512)
                nc.scalar.copy(dtin_sb[:, sl], psd[tt][0:DTRANK, :])
                bct = pC.tile([112, 512], f32, tag="bct")
                nc.scalar.copy(bct[64:80, :], psd[tt][64:80, :])
                nc.scalar.copy(bct[96:112, :], psd[tt][96:112, :])
                nc.sync.dma_start(bc_d[0, :, sl], bct[64:80, :])
                nc.sync.dma_start(bc_d[1, :, sl], bct[96:112, :])

        # ---------- phase D ----------
        with tc.tile_pool(name="pbc", bufs=1) as pBC, \
             tc.tile_pool(name="pbig", bufs=2) as pBig, \
             tc.tile_pool(name="pu", bufs=1) as pU, \
             tc.tile_pool(name="ph", bufs=1) as pH, \
             tc.tile_pool(name="psm", bufs=1) as pS, \
             tc.tile_pool(name="py", bufs=1) as pY, \
             tc.tile_pool(name="pw", bufs=2) as pW, \
             tc.tile_pool(name="pln", bufs=1) as pLN:
            for blk in range(NBLK):
                tsl = slice(blk * TB, (blk + 1) * TB)
                Bb = [pBC.tile([128, GS * TB], bf16, tag=f"Bb{g}", name=f"Bb{blk}_{g}") for g in range(NGRP)]
                Cb = [pBC.tile([128, GS * TB], bf16, tag=f"Cb{g}", name=f"Cb{blk}_{g}") for g in range(NGRP)]
                for g in range(NGRP):
                    gsl = slice(g * GS, (g + 1) * GS)
                    nc.gpsimd.dma_start(Bb[g][:], bc_d[0, gsl, tsl].partition_broadcast(128))
                    nc.gpsimd.dma_start(Cb[g][:], bc_d[1, gsl, tsl].partition_broadcast(128))

                ygs = []
                with tc.tile_pool(name="pd_ps", bufs=2, space="PSUM") as pD_ps:
                    for i in range(NCH):
                        ps = pD_ps.tile([128, TB], f32, tag="argps")
                        nc.tensor.matmul(ps[:], dtw[i][:], dtin_sb[:, tsl],
                                         start=True, stop=True)
                        e_t = pS.tile([128, TB], f32, tag="et")
                        nc.scalar.activation(e_t[:], ps[:], AF.Exp, bias=dtpb[:, i:i + 1])
                        dt_t = pS.tile([128, TB], f32, tag="dtt", bufs=2)
                        nc.scalar.activation(dt_t[:], e_t[:], AF.Ln, bias=1.0)
                        xc_t = pS.tile([128, TB], f32, tag="xctd", bufs=2)
                        nc.sync.dma_start(xc_t[:], xc_d[i, :, tsl])
                        zs_t = pS.tile([128, TB], f32, tag="zstd", bufs=2)
                        nc.sync.dma_start(zs_t[:], zs_d[i, :, tsl])
                        dtx = pS.tile([128, TB], f32, tag="dtx")
                        nc.vector.tensor_tensor(dtx[:], dt_t[:], xc_t[:], OP.mult)

                        y_acc = pS.tile([128, TB], f32, tag="yacc")
                        for g in range(NGRP):
                            csl = slice(g * GS, (g + 1) * GS)
                            a8 = pBig.tile([128, GS * TB], f32, tag="a8")
                            for n in range(GS):
                                nn_ = g * GS + n
                                nc.scalar.activation(a8[:, n * TB:(n + 1) * TB], dt_t[:],
                                                     AF.Exp, scale=A_t[i][:, nn_:nn_ + 1])
                            u8 = pU.tile([128, GS * TB], f32, tag="u8")
                            dtxb = dtx[:][:, None, :].broadcast_to([128, GS, TB])
                            nc.vector.tensor_tensor(
                                u8[:], dtxb,
                                Bb[g][:].rearrange("p (s t) -> p s t", s=GS), OP.mult)
                            a8v = a8[:].rearrange("p (s t) -> p s t", s=GS)
                            u8v = u8[:].rearrange("p (s t) -> p s t", s=GS)
                            if blk > 0:
                                tmp = pS.tile([128, GS], f32, tag="cytmp")
                                nc.vector.tensor_tensor(
                                    tmp[:], a8v[:, :, 0:1].squeeze(),
                                    cys[i][:, csl], OP.mult)
                                nc.vector.tensor_tensor(
                                    u8v[:, :, 0:1].squeeze(),
                                    u8v[:, :, 0:1].squeeze(), tmp[:], OP.add)
                            nc.vector.memset(a8v[:, :, 0:1], 0.0)
                            h8 = pH.tile([128, GS * TB], f32, tag="h8")
                            nc.vector.tensor_tensor_scan(h8[:], a8[:], u8[:], 0.0,
                                                         OP.mult, OP.add)
                            if blk < NBLK - 1:
                                nc.vector.tensor_copy(
                                    cys[i][:, csl],
                                    h8[:].rearrange("p (s t) -> p s t",
                                                    s=GS)[:, :, TB - 1:TB].squeeze())
                            prod = pBig.tile([128, GS * TB], bf16, tag="prodb")
                            nc.vector.tensor_tensor(prod[:], h8[:], Cb[g][:], OP.mult)
                            # pairwise tree over the 8 sections (contiguous adds
                            # stay in the 2x bf16 perf mode; strided reduce can't)
                            nc.vector.tensor_tensor(prod[:, 0:4 * TB], prod[:, 0:4 * TB],
                                                    prod[:, 4 * TB:8 * TB], OP.add)
                            nc.vector.tensor_tensor(prod[:, 0:2 * TB], prod[:, 0:2 * TB],
                                                    prod[:, 2 * TB:4 * TB], OP.add)
                            if g == 0:
                                nc.vector.tensor_tensor(y_acc[:], prod[:, 0:TB],
                                                        prod[:, TB:2 * TB], OP.add)
                            else:
                                y2 = pS.tile([128, TB], f32, tag="y2")
                                nc.vector.tensor_tensor(y2[:], prod[:, 0:TB],
                                                        prod[:, TB:2 * TB], OP.add)
                                nc.vector.tensor_tensor(y_acc[:], y_acc[:], y2[:],
                                                        OP.add)
                        nc.vector.scalar_tensor_tensor(
                            y_acc[:], xc_t[:], dDc[:, i:i + 1], y_acc[:],
                            op0=OP.mult, op1=OP.add)
                        yg = pY.tile([128, TB], f32, tag=f"yg{i}")
                        nc.vector.tensor_tensor(yg[:], y_acc[:], zs_t[:], OP.mult)
                        ygs.append(yg)

                # out_proj + residual + LayerNorm for this block
                with tc.tile_pool(name="po_ps", bufs=1, space="PSUM") as pO_ps:
                    ops = [(pO_ps.tile([128, 512], f32, tag=f"op1_{t4}", name=f"op1_{blk}_{t4}"),
                            pO_ps.tile([128, 256], f32, tag=f"op2_{t4}", name=f"op2_{blk}_{t4}"))
                           for t4 in range(TB // 128)]
                    for i in range(NCH):
                        wt = pW.tile([128, DIM], f32, tag="wout")
                        nc.sync.dma_start(wt[:], wout_d[i * 128:(i + 1) * 128, :])
                        for t4 in range(TB // 128):
                            lhs = ygs[i][:, t4 * 128:(t4 + 1) * 128]
                            nc.tensor.matmul(ops[t4][0][:], lhs, wt[:, 0:512],
                                             start=(i == 0), stop=(i == NCH - 1))
                            nc.tensor.matmul(ops[t4][1][:], lhs, wt[:, 512:768],
                                             start=(i == 0), stop=(i == NCH - 1))
                    for t4 in range(TB // 128):
                        trow = blk * TB + t4 * 128
                        xres = pLN.tile([128, DIM], f32, tag="xres")
                        nc.sync.dma_start(xres[:], x_d[trow:trow + 128, :])
                        r = pLN.tile([128, DIM], f32, tag="r")
                        nc.vector.scalar_tensor_tensor(
                            r[:, 0:512], ops[t4][0][:], 0.1, xres[:, 0:512],
                            op0=OP.mult, op1=OP.add)
                        nc.vector.scalar_tensor_tensor(
                            r[:, 512:768], ops[t4][1][:], 0.1, xres[:, 512:768],
                            op0=OP.mult, op1=OP.add)
                        mu = pLN.tile([128, 1], f32, tag="mu")
                        nc.vector.tensor_reduce(mu[:], r[:], AX.X, OP.add)
                        nc.scalar.mul(mu[:], mu[:], 1.0 / DIM)
                        nc.vector.tensor_scalar(r[:], r[:], mu[:], None,
                                                op0=OP.subtract)
                        sq = pLN.tile([128, DIM], f32, tag="sq")
                        nc.scalar.activation(sq[:], r[:], AF.Square)
                        var = pLN.tile([128, 1], f32, tag="var")
                        nc.vector.tensor_reduce(var[:], sq[:], AX.X, OP.add)
                        lnv = pLN.tile([128, 1], f32, tag="lnv")
                        nc.scalar.activation(lnv[:], var[:], AF.Ln, scale=1.0 / DIM,
                                             bias=epsc[:])
                        rstd = pLN.tile([128, 1], f32, tag="rstd")
                        nc.scalar.activation(rstd[:], lnv[:], AF.Exp, scale=-0.5)
                        nc.vector.tensor_scalar(r[:], r[:], rstd[:], None, op0=OP.mult)
                        nc.vector.tensor_tensor(sq[:], r[:], lngb[:], OP.mult)
                        nc.vector.tensor_tensor(sq[:], sq[:], lnbb[:], OP.add)
                        nc.sync.dma_start(out_d[trow:trow + 128, :], sq[:])

    nc.compile()
    return nc


def _get_nc():
    if "nc" not in _CACHE:
        _CACHE["nc"] = _build()
    return _CACHE["nc"]


def kernel(**inputs):
    from concourse.bass_utils import run_bass_kernel_spmd

    nc = _get_nc()
    shared = {k: np.ascontiguousarray(np.asarray(inputs[k], np.float32))
              for k in ("sp_w1", "sp_b1", "sp_w2", "sp_b2", "in_proj_w", "conv_w",
                        "conv_b", "x_proj_w", "dt_proj_w", "dt_proj_b", "A_log",
                        "D", "out_proj_w", "ln_g", "ln_b")}
    x = np.asarray(inputs["x"], np.float32)
    sal = np.asarray(inputs["saliency_score"], np.float32)
    in_maps = []
    for c in range(B):
        m = dict(shared)
        m["x"] = np.ascontiguousarray(x[c])
        m["sal"] = np.ascontiguousarray(sal[c])
        in_maps.append(m)
    res = run_bass_kernel_spmd(nc, in_maps, core_ids=list(range(B)))
    out = np.stack([res.results[c]["out"] for c in range(B)], axis=0)
    return out



# revision 8
# speedup vs baseline: 1.3711x; 1.3711x over previous
"""ContentAwareMambaFilter Trainium2 kernel (v2, bf16 datapath).

Data-parallel over batch: 8 NeuronCores, one batch row each. Takes full
(unsharded) inputs, returns the full output; per-core slicing happens in
kernel(). The Bass program is built and compiled once, then cached.

Key design points vs v1:
  - All matmuls in bf16 (fp32 PE is 4 cycles/row, bf16 is 1).
  - Depthwise conv runs on the PE via diag(w_k) matmuls (off the DVE).
  - Whole scan datapath in bf16 (DVE 2x mode for the big elementwise
    ops; scan rate itself is dtype-independent). Scan state is fp32
    internally so bf16 operands only round per-step inputs.
  - One scan instruction per (block, chunk) covering all 16 states;
    section boundaries broken by zeroing a8 at section starts and
    injecting the carry into u8.
  - dt (softplus) computed for all t in a separate pass: Exp-run then
    Ln-run, so ACT table loads happen O(1) times, not per iteration.
  - a8 = exp(A_n * dt) stays on ACT (16 ops per block-chunk).
  - out_proj + residual + LayerNorm in a final phase; residual is
    injected into PSUM via identity matmul; mean/var via bn_stats;
    rsqrt batched over all tiles (DVE reciprocal + one ACT Sqrt).
"""

import numpy as np

B = 8
L = 2048
DIM = 768
DSTATE = 16
DCONV = 4
DINNER = 1536
DTRANK = 48

NCH = DINNER // 128          # 12 channel chunks
CCH = DIM // 128             # 6 dim chunks
TB = 512                     # scan time block
NBLK = L // TB
NTT = L // 512               # matmul t tiles
NTC = L // 128               # out/LN row tiles
EPS = 1e-5

_CACHE = {}


def _build():
    from contextlib import ExitStack
    import concourse.bacc as bacc
    import concourse.tile as tile
    import concourse.mybir as mybir
    from concourse.masks import make_identity

    f32 = mybir.dt.float32
    bf16 = mybir.dt.bfloat16
    AF = mybir.ActivationFunctionType
    OP = mybir.AluOpType
    AX = mybir.AxisListType

    nc = bacc.Bacc("TRN2", target_bir_lowering=False, debug=False)

    x_d = nc.dram_tensor("x", [L, DIM], f32, kind="ExternalInput").ap()
    sal_d = nc.dram_tensor("sal", [L, 1], f32, kind="ExternalInput").ap()
    spw1_d = nc.dram_tensor("sp_w1", [1, DIM // 4], f32, kind="ExternalInput").ap()
    spb1_d = nc.dram_tensor("sp_b1", [DIM // 4], f32, kind="ExternalInput").ap()
    spw2_d = nc.dram_tensor("sp_w2", [DIM // 4, 2 * DIM], f32, kind="ExternalInput").ap()
    spb2_d = nc.dram_tensor("sp_b2", [2 * DIM], f32, kind="ExternalInput").ap()
    win_d = nc.dram_tensor("in_proj_w", [DIM, 2 * DINNER], f32, kind="ExternalInput").ap()
    wcv_d = nc.dram_tensor("conv_w", [DINNER, DCONV], f32, kind="ExternalInput").ap()
    cvb_d = nc.dram_tensor("conv_b", [DINNER], f32, kind="ExternalInput").ap()
    wxp_d = nc.dram_tensor("x_proj_w", [DINNER, DTRANK + 2 * DSTATE], f32, kind="ExternalInput").ap()
    wdt_d = nc.dram_tensor("dt_proj_w", [DTRANK, DINNER], f32, kind="ExternalInput").ap()
    dtb_d = nc.dram_tensor("dt_proj_b", [DINNER], f32, kind="ExternalInput").ap()
    alog_d = nc.dram_tensor("A_log", [DINNER, DSTATE], f32, kind="ExternalInput").ap()
    dD_d = nc.dram_tensor("D", [DINNER], f32, kind="ExternalInput").ap()
    wout_d = nc.dram_tensor("out_proj_w", [DINNER, DIM], f32, kind="ExternalInput").ap()
    lng_d = nc.dram_tensor("ln_g", [DIM], f32, kind="ExternalInput").ap()
    lnb_d = nc.dram_tensor("ln_b", [DIM], f32, kind="ExternalInput").ap()
    out_d = nc.dram_tensor("out", [L, DIM], f32, kind="ExternalOutput").ap()

    xc_d = nc.dram_tensor("xc_scr", [NCH, 128, L], bf16).ap()
    zs_d = nc.dram_tensor("zs_scr", [NCH, 128, L], bf16).ap()
    dt_d = nc.dram_tensor("dt_scr", [NCH, 128, L], bf16).ap()
    bc_d = nc.dram_tensor("bc_scr", [2, DSTATE, L], bf16).ap()

    with tile.TileContext(nc) as tc, ExitStack() as ctx:
        # ---------- long-lived constants ----------
        consts = ctx.enter_context(tc.tile_pool(name="consts", bufs=1))

        identb = consts.tile([128, 128], bf16, tag="identb")
        make_identity(nc, identb[:])

        A_t = []
        for i in range(NCH):
            al = consts.tile([128, DSTATE], f32, tag=f"alog{i}")
            nc.sync.dma_start(al[:], alog_d[i * 128:(i + 1) * 128, :])
            at = consts.tile([128, DSTATE], f32, tag=f"at{i}")
            nc.scalar.activation(at[:], al[:], AF.Exp)
            nc.vector.tensor_scalar_mul(at[:], at[:], -1.0)
            A_t.append(at)

        def col_per_chunk(src_vec, name):
            t = consts.tile([128, NCH], f32, tag=name)
            nc.sync.dma_start(
                t[:], src_vec.rearrange("(i p) -> i p", p=128).transpose([1, 0]))
            return t

        dtpb = col_per_chunk(dtb_d, "dtpb")
        dDc = col_per_chunk(dD_d, "dDc")
        cvbc = col_per_chunk(cvb_d, "cvbc")
        lngb = consts.tile([128, DIM], bf16, tag="lngb")
        nc.gpsimd.dma_start(lngb[:], lng_d.partition_broadcast(128))
        lnbb = consts.tile([128, DIM], f32, tag="lnbb")
        nc.sync.dma_start(lnbb[:], lnb_d.partition_broadcast(128))

        # conv diag weights: diag(w_k) per chunk, bf16
        wcvc = consts.tile([128, NCH * DCONV], f32, tag="wcvc")
        nc.sync.dma_start(
            wcvc[:], wcv_d.rearrange("(i p) k -> i p k", p=128).transpose([1, 0, 2]))
        cdiag = []
        for i in range(NCH):
            row = []
            for k in range(DCONV):
                t = consts.tile([128, 128], bf16, tag=f"cd{i}_{k}")
                nc.vector.tensor_scalar(
                    t[:], identb[:], wcvc[:, i * DCONV + k:i * DCONV + k + 1],
                    None, op0=OP.mult)
                row.append(t)
            cdiag.append(row)

        dtw = []
        for i in range(NCH):
            t = consts.tile([DTRANK, 128], bf16, tag=f"dtw{i}")
            nc.gpsimd.dma_start(t[:], wdt_d[:, i * 128:(i + 1) * 128])
            dtw.append(t)

        xpw = []
        for i in range(NCH):
            t = consts.tile([128, 112], bf16, tag=f"xpw{i}")
            nc.vector.memset(t[:], 0.0)
            isl = slice(i * 128, (i + 1) * 128)
            nc.gpsimd.dma_start(t[:, 0:DTRANK], wxp_d[isl, 0:DTRANK])
            nc.gpsimd.dma_start(t[:, 64:80], wxp_d[isl, DTRANK:DTRANK + DSTATE])
            nc.gpsimd.dma_start(t[:, 96:112], wxp_d[isl, DTRANK + DSTATE:])
            xpw.append(t)

        # out_proj weights, pre-scaled by 0.1 (residual r = x + 0.1*mamba)
        woutb = []
        with tc.tile_pool(name="wstage", bufs=2) as wstage:
            for i in range(NCH):
                wf = wstage.tile([128, DIM], f32, tag="woutf")
                nc.sync.dma_start(wf[:], wout_d[i * 128:(i + 1) * 128, :])
                t = consts.tile([128, DIM], bf16, tag=f"woutb{i}")
                nc.scalar.activation(t[:], wf[:], AF.Copy, scale=0.1)
                woutb.append(t)

        dtin_sb = consts.tile([DTRANK, L], bf16, tag="dtin")
        epsc = consts.tile([128, 1], f32, tag="epsc")
        nc.vector.memset(epsc[:], EPS)
        cys = [consts.tile([128, DSTATE], f32, tag=f"cy{i}", name=f"cy{i}")
               for i in range(NCH)]

        # ---------- phase A: FiLM + transpose ----------
        with tc.tile_pool(name="xmod", bufs=1) as xmod_pool:
            xmod = [xmod_pool.tile([128, L], bf16, tag=f"xm{cc}", name=f"xm{cc}")
                    for cc in range(CCH)]

            with tc.tile_pool(name="pa", bufs=2) as pA, \
                 tc.tile_pool(name="pa_c", bufs=1) as pAc, \
                 tc.tile_pool(name="pa_ps", bufs=2, space="PSUM") as pA_ps, \
                 tc.tile_pool(name="pa_pst", bufs=2, space="PSUM") as pA_pst:
                ones96 = pAc.tile([1, 96], bf16, tag="ones96")
                nc.vector.memset(ones96[:], 1.0)
                w1c = pAc.tile([96, 2], f32, tag="w1c")
                nc.sync.dma_start(
                    w1c[:], spw1_d.rearrange("o (g j) -> o g j", g=2).squeeze(0).transpose([1, 0]))
                b1c = pAc.tile([96, 2], f32, tag="b1c")
                nc.sync.dma_start(b1c[:], spb1_d.rearrange("(g j) -> g j", g=2).transpose([1, 0]))
                spb2c = pAc.tile([128, 12], f32, tag="spb2")
                nc.sync.dma_start(
                    spb2c[:], spb2_d.rearrange("(i p) -> i p", p=128).transpose([1, 0]))
                w2c = []
                for kc in range(2):
                    row = []
                    for m in range(12):
                        t = pAc.tile([96, 128], bf16, tag=f"w2c{kc}_{m}")
                        nc.gpsimd.dma_start(
                            t[:], spw2_d[kc * 96:(kc + 1) * 96, m * 128:(m + 1) * 128])
                        row.append(t)
                    w2c.append(row)

                # saliency broadcast + FiLM hidden layer
                sal_sb = pAc.tile([1, L], bf16, tag="salsb")
                nc.gpsimd.dma_start(sal_sb[:], sal_d.transpose([1, 0]))
                h2 = [pAc.tile([96, L], bf16, tag=f"h2_{kc}", name=f"h2_{kc}")
                      for kc in range(2)]
                for kc in range(2):
                    for tt in range(NTT):
                        ps = pA_ps.tile([96, 512], f32, tag="salps")
                        nc.tensor.matmul(ps[:], ones96[:],
                                         sal_sb[:, tt * 512:(tt + 1) * 512],
                                         start=True, stop=True)
                        nc.scalar.activation(h2[kc][:, tt * 512:(tt + 1) * 512], ps[:],
                                             AF.Relu, scale=w1c[:, kc:kc + 1],
                                             bias=b1c[:, kc:kc + 1])

                # x transpose -> xmod tiles hold xT (bf16)
                for cc in range(CCH):
                    for tcn in range(L // 128):
                        xt_in = pA.tile([128, 128], bf16, tag="xtin")
                        nc.gpsimd.dma_start(
                            xt_in[:], x_d[tcn * 128:(tcn + 1) * 128, cc * 128:(cc + 1) * 128])
                        ps = pA_pst.tile([128, 128], bf16, tag="xtps")
                        nc.tensor.transpose(ps[:], xt_in[:], identb[:])
                        nc.scalar.copy(xmod[cc][:, tcn * 128:(tcn + 1) * 128], ps[:])

                # FiLM affine + modulation: x_mod = x*tanh(g) + (x + beta)
                for cc in range(CCH):
                    for tt in range(NTT):
                        sl = slice(tt * 512, (tt + 1) * 512)
                        psg = pA_ps.tile([128, 512], f32, tag="affg")
                        for kc in range(2):
                            nc.tensor.matmul(psg[:], w2c[kc][cc][:], h2[kc][:, sl],
                                             start=(kc == 0), stop=(kc == 1))
                        tg = pA.tile([128, 512], bf16, tag="tg")
                        nc.scalar.activation(tg[:], psg[:], AF.Tanh,
                                             bias=spb2c[:, cc:cc + 1])
                        psb = pA_ps.tile([128, 512], f32, tag="affb")
                        for kc in range(2):
                            nc.tensor.matmul(psb[:], w2c[kc][cc + 6][:], h2[kc][:, sl],
                                             start=(kc == 0), stop=False)
                        # inject x so bt = x + beta
                        nc.tensor.matmul(psb[:], identb[:], xmod[cc][:, sl],
                                         start=False, stop=True)
                        bt = pA.tile([128, 512], bf16, tag="bt")
                        nc.scalar.activation(bt[:], psb[:], AF.Identity,
                                             bias=spb2c[:, cc + 6:cc + 7])
                        t1 = pA.tile([128, 512], bf16, tag="t1")
                        nc.vector.tensor_tensor(t1[:], xmod[cc][:, sl], tg[:], OP.mult)
                        nc.vector.tensor_tensor(xmod[cc][:, sl], t1[:], bt[:], OP.add)

            # ---------- phase B1: in_proj -> xin tiles + silu(z) ----------
            with tc.tile_pool(name="xin_pool", bufs=1) as xin_pool:
                xins = [xin_pool.tile([128, L + 3], bf16, tag=f"xin{i}", name=f"xin{i}")
                        for i in range(NCH)]
                with tc.tile_pool(name="pb", bufs=2) as pB, \
                     tc.tile_pool(name="pb_w", bufs=3) as pB_w, \
                     tc.tile_pool(name="pb_ps", bufs=2, space="PSUM") as pB_ps:
                    for m in range(24):
                        psl = pB_ps.tile([128, L], f32, tag="ipp", name=f"ipp{m}")
                        for cc in range(CCH):
                            wt = pB_w.tile([128, 128], bf16, tag="wstage")
                            nc.gpsimd.dma_start(
                                wt[:], win_d[cc * 128:(cc + 1) * 128, m * 128:(m + 1) * 128])
                            for tt in range(NTT):
                                nc.tensor.matmul(psl[:, tt * 512:(tt + 1) * 512],
                                                 wt[:],
                                                 xmod[cc][:, tt * 512:(tt + 1) * 512],
                                                 start=(cc == 0), stop=(cc == CCH - 1))
                        if m >= 12:
                            i = m - 12
                            zt = pB.tile([128, L], bf16, tag="ztile")
                            for tt in range(NTT):
                                nc.scalar.activation(zt[:, tt * 512:(tt + 1) * 512],
                                                     psl[:, tt * 512:(tt + 1) * 512],
                                                     AF.Silu)
                            nc.sync.dma_start(zs_d[i], zt[:])
                        else:
                            i = m
                            nc.vector.memset(xins[i][:, 0:3], 0.0)
                            for tt in range(NTT):
                                nc.scalar.copy(
                                    xins[i][:, 3 + tt * 512:3 + (tt + 1) * 512],
                                    psl[:, tt * 512:(tt + 1) * 512])

                # ---------- phase B2: conv on PE + silu ----------
                with tc.tile_pool(name="pc2", bufs=2) as pC2, \
                     tc.tile_pool(name="pc2_ps", bufs=4, space="PSUM") as pC2_ps:
                    for i in range(NCH):
                        xct = pC2.tile([128, L], bf16, tag="xct")
                        for tt in range(NTT):
                            cps = pC2_ps.tile([128, 512], f32, tag="cps")
                            for k in range(DCONV):
                                nc.tensor.matmul(
                                    cps[:], cdiag[i][k][:],
                                    xins[i][:, k + tt * 512:k + (tt + 1) * 512],
                                    start=(k == 0), stop=(k == DCONV - 1))
                            nc.scalar.activation(xct[:, tt * 512:(tt + 1) * 512],
                                                 cps[:], AF.Silu,
                                                 bias=cvbc[:, i:i + 1])
                        nc.sync.dma_start(xc_d[i], xct[:])

        # ---------- phase C: x_proj ----------
        with tc.tile_pool(name="pc", bufs=2) as pC, \
             tc.tile_pool(name="pc_ps", bufs=1, space="PSUM") as pC_ps:
            psd = pC_ps.tile([112, L], f32, tag="psd")
            for i in range(NCH):
                xci = pC.tile([128, L], bf16, tag="xcld")
                nc.sync.dma_start(xci[:], xc_d[i])
                for tt in range(NTT):
                    nc.tensor.matmul(psd[:, tt * 512:(tt + 1) * 512], xpw[i][:],
                                     xci[:, tt * 512:(tt + 1) * 512],
                                     start=(i == 0), stop=(i == NCH - 1))
            nc.scalar.copy(dtin_sb[:], psd[0:DTRANK, :])
            bcb = pC.tile([DSTATE, L], bf16, tag="bcb")
            nc.scalar.copy(bcb[:], psd[64:80, :])
            nc.sync.dma_start(bc_d[0], bcb[:])
            bcc = pC.tile([DSTATE, L], bf16, tag="bcc")
            nc.scalar.copy(bcc[:], psd[96:112, :])
            nc.sync.dma_start(bc_d[1], bcc[:])

        # ---------- phase C2: dt = softplus(dtin @ dt_proj_w + b) ----------
        with tc.tile_pool(name="pdt", bufs=2) as pDT, \
             tc.tile_pool(name="pdt_ps", bufs=2, space="PSUM") as pDT_ps:
            evs = []
            for i in range(NCH):
                ps = pDT_ps.tile([128, L], f32, tag="dtps", name=f"dtps{i}")
                for tt in range(NTT):
                    nc.tensor.matmul(ps[:, tt * 512:(tt + 1) * 512], dtw[i][:],
                                     dtin_sb[:, tt * 512:(tt + 1) * 512],
                                     start=True, stop=True)
                ev = pDT.tile([128, L], bf16, tag=f"ev{i % 4}", name=f"ev{i}")
                nc.scalar.activation(ev[:], ps[:], AF.Exp, bias=dtpb[:, i:i + 1])
                evs.append(ev)
                if i >= 3:
                    j = i - 3
                    dtt = pDT.tile([128, L], bf16, tag="dtt")
                    nc.scalar.activation(dtt[:], evs[j][:], AF.Ln, bias=1.0)
                    nc.sync.dma_start(dt_d[j], dtt[:])
            for j in range(NCH - 3, NCH):
                dtt = pDT.tile([128, L], bf16, tag="dtt")
                nc.scalar.activation(dtt[:], evs[j][:], AF.Ln, bias=1.0)
                nc.sync.dma_start(dt_d[j], dtt[:])

        # ---------- phase D: selective scan ----------
        with tc.tile_pool(name="pyg", bufs=1) as pYG:
            ygs = [pYG.tile([128, L], bf16, tag=f"yg{i}", name=f"yg{i}")
                   for i in range(NCH)]
            pD_stack = ExitStack()
            pBC = pD_stack.enter_context(tc.tile_pool(name="pbc", bufs=1))
            pS = pD_stack.enter_context(tc.tile_pool(name="pd_s", bufs=2))
            pW = pD_stack.enter_context(tc.tile_pool(name="pd_w", bufs=1))
            for blk in range(NBLK):
                tsl = slice(blk * TB, (blk + 1) * TB)
                Bb = pBC.tile([128, DSTATE * TB], bf16, tag="Bb", name=f"Bb{blk}")
                Cb = pBC.tile([128, DSTATE * TB], bf16, tag="Cb", name=f"Cb{blk}")
                nc.gpsimd.dma_start(Bb[:], bc_d[0, :, tsl].partition_broadcast(128))
                nc.gpsimd.dma_start(Cb[:], bc_d[1, :, tsl].partition_broadcast(128))

                for i in range(NCH):
                    dt_t = pS.tile([128, TB], bf16, tag="dtt")
                    nc.sync.dma_start(dt_t[:], dt_d[i, :, tsl])
                    xc_t = pS.tile([128, TB], bf16, tag="xctd")
                    nc.sync.dma_start(xc_t[:], xc_d[i, :, tsl])
                    zs_t = pS.tile([128, TB], bf16, tag="zstd")
                    nc.sync.dma_start(zs_t[:], zs_d[i, :, tsl])

                    # a8[n] = exp(A[:,n] * dt)  (16 ACT ops)
                    a8 = pW.tile([128, DSTATE * TB], bf16, tag="a8")
                    for n in range(DSTATE):
                        nc.scalar.activation(a8[:, n * TB:(n + 1) * TB], dt_t[:],
                                             AF.Exp, scale=A_t[i][:, n:n + 1])

                    dtx = pS.tile([128, TB], bf16, tag="dtx")
                    nc.vector.tensor_tensor(dtx[:], dt_t[:], xc_t[:], OP.mult)
                    u8 = pW.tile([128, DSTATE * TB], bf16, tag="u8")
                    nc.vector.tensor_tensor(
                        u8[:], dtx[:][:, None, :].broadcast_to([128, DSTATE, TB]),
                        Bb[:].rearrange("p (s t) -> p s t", s=DSTATE), OP.mult)

                    a8v = a8[:].rearrange("p (s t) -> p s t", s=DSTATE)
                    u8v = u8[:].rearrange("p (s t) -> p s t", s=DSTATE)
                    if blk > 0:
                        tmp = pS.tile([128, DSTATE], f32, tag="cytmp")
                        nc.vector.tensor_tensor(
                            tmp[:], a8v[:, :, 0:1].squeeze(), cys[i][:], OP.mult)
                        nc.vector.tensor_tensor(
                            u8v[:, :, 0:1].squeeze(),
                            u8v[:, :, 0:1].squeeze(), tmp[:], OP.add)
                    nc.vector.memset(a8v[:, :, 0:1], 0.0)

                    h8 = pW.tile([128, DSTATE * TB], bf16, tag="h8")
                    nc.vector.tensor_tensor_scan(h8[:], a8[:], u8[:], 0.0,
                                                 OP.mult, OP.add)
                    if blk < NBLK - 1:
                        nc.vector.tensor_copy(
                            cys[i][:],
                            h8[:].rearrange("p (s t) -> p s t",
                                            s=DSTATE)[:, :, TB - 1:TB].squeeze())

                    # y = sum_n C_n * h_n  (mult + pairwise tree, all bf16);
                    # u8 is dead after the scan, reuse its buffer for prod
                    prod = pW.tile([128, DSTATE * TB], bf16, tag="u8",
                                   name=f"prod{blk}_{i}")
                    nc.vector.tensor_tensor(prod[:], h8[:], Cb[:], OP.mult)
                    nc.vector.tensor_tensor(prod[:, 0:8 * TB], prod[:, 0:8 * TB],
                                            prod[:, 8 * TB:16 * TB], OP.add)
                    nc.vector.tensor_tensor(prod[:, 0:4 * TB], prod[:, 0:4 * TB],
                                            prod[:, 4 * TB:8 * TB], OP.add)
                    nc.vector.tensor_tensor(prod[:, 0:2 * TB], prod[:, 0:2 * TB],
                                            prod[:, 2 * TB:4 * TB], OP.add)
                    y1 = pS.tile([128, TB], bf16, tag="y1")
                    nc.vector.tensor_tensor(y1[:], prod[:, 0:TB],
                                            prod[:, TB:2 * TB], OP.add)
                    # y += D*xc ; gate with silu(z)
                    y2 = pS.tile([128, TB], bf16, tag="y2")
                    nc.vector.scalar_tensor_tensor(
                        y2[:], xc_t[:], dDc[:, i:i + 1], y1[:],
                        op0=OP.mult, op1=OP.add)
                    nc.vector.tensor_tensor(ygs[i][:, tsl], y2[:], zs_t[:], OP.mult)

            pD_stack.close()

            # ---------- phase E: out_proj + residual + LayerNorm ----------
            with tc.tile_pool(name="pe", bufs=2) as pE, \
                 tc.tile_pool(name="pe_r", bufs=1) as pER, \
                 tc.tile_pool(name="pe_ps", bufs=2, space="PSUM") as pE_ps:
                rts = [pER.tile([128, DIM], bf16, tag=f"rt{tcn}", name=f"rt{tcn}")
                       for tcn in range(NTC)]
                mv = pER.tile([128, 2 * NTC], f32, tag="mv")
                for tcn in range(NTC):
                    trow = tcn * 128
                    xres = pE.tile([128, DIM], bf16, tag="xres")
                    nc.gpsimd.dma_start(xres[:], x_d[trow:trow + 128, :])
                    ps1 = pE_ps.tile([128, 512], f32, tag="op1")
                    ps2 = pE_ps.tile([128, 256], f32, tag="op2")
                    nc.tensor.matmul(ps1[:], identb[:], xres[:, 0:512],
                                     start=True, stop=False)
                    nc.tensor.matmul(ps2[:], identb[:], xres[:, 512:768],
                                     start=True, stop=False)
                    for i in range(NCH):
                        lhs = ygs[i][:, trow:trow + 128]
                        nc.tensor.matmul(ps1[:], lhs, woutb[i][:, 0:512],
                                         start=False, stop=(i == NCH - 1))
                        nc.tensor.matmul(ps2[:], lhs, woutb[i][:, 512:768],
                                         start=False, stop=(i == NCH - 1))
                    nc.scalar.copy(rts[tcn][:, 0:512], ps1[:])
                    nc.scalar.copy(rts[tcn][:, 512:768], ps2[:])
                    st6 = pE.tile([128, 12], f32, tag="st6")
                    nc.vector.bn_stats(st6[:, 0:6], rts[tcn][:, 0:512])
                    nc.vector.bn_stats(st6[:, 6:12], rts[tcn][:, 512:768])
                    nc.vector.bn_aggr(mv[:, 2 * tcn:2 * tcn + 2],
                                      st6[:].rearrange("p (g s) -> p g s", g=2))

                # rstd for all tiles: 1/sqrt(var+eps), batched
                vare = pER.tile([128, NTC], f32, tag="vare")
                nc.vector.tensor_scalar(
                    vare[:], mv[:].rearrange("p (t s) -> p t s", s=2)[:, :, 1:2].squeeze(),
                    epsc[:], None, op0=OP.add)
                vrec = pER.tile([128, NTC], f32, tag="vrec")
                nc.vector.reciprocal(vrec[:], vare[:])
                rstd = pER.tile([128, NTC], f32, tag="rstd")
                nc.scalar.activation(rstd[:], vrec[:], AF.Sqrt)

                for tcn in range(NTC):
                    trow = tcn * 128
                    t1 = pE.tile([128, DIM], bf16, tag="t1")
                    nc.vector.tensor_scalar(
                        t1[:], rts[tcn][:], mv[:, 2 * tcn:2 * tcn + 1],
                        rstd[:, tcn:tcn + 1], op0=OP.subtract, op1=OP.mult)
                    t2 = pE.tile([128, DIM], bf16, tag="t2")
                    nc.vector.tensor_tensor(t2[:], t1[:], lngb[:], OP.mult)
                    of = pE.tile([128, DIM], f32, tag="of")
                    nc.vector.tensor_tensor(of[:], t2[:], lnbb[:], OP.add)
                    nc.sync.dma_start(out_d[trow:trow + 128, :], of[:])

    nc.compile()
    return nc


def _get_nc():
    if "nc" not in _CACHE:
        _CACHE["nc"] = _build()
    return _CACHE["nc"]


def kernel(**inputs):
    from concourse.bass_utils import run_bass_kernel_spmd

    nc = _get_nc()
    shared = {k: np.ascontiguousarray(np.asarray(inputs[k], np.float32))
              for k in ("sp_w1", "sp_b1", "sp_w2", "sp_b2", "in_proj_w", "conv_w",
                        "conv_b", "x_proj_w", "dt_proj_w", "dt_proj_b", "A_log",
                        "D", "out_proj_w", "ln_g", "ln_b")}
    x = np.asarray(inputs["x"], np.float32)
    sal = np.asarray(inputs["saliency_score"], np.float32)
    in_maps = []
    for c in range(B):
        m = dict(shared)
        m["x"] = np.ascontiguousarray(x[c])
        m["sal"] = np.ascontiguousarray(sal[c])
        in_maps.append(m)
    res = run_bass_kernel_spmd(nc, in_maps, core_ids=list(range(B)))
    out = np.stack([res.results[c]["out"] for c in range(B)], axis=0)
    return out


# revision 15
# speedup vs baseline: 1.4243x; 1.0388x over previous
"""ContentAwareMambaFilter Trainium2 kernel (v2, bf16 datapath).

Data-parallel over batch: 8 NeuronCores, one batch row each. Takes full
(unsharded) inputs, returns the full output; per-core slicing happens in
kernel(). The Bass program is built and compiled once, then cached.

Key design points vs v1:
  - All matmuls in bf16 (fp32 PE is 4 cycles/row, bf16 is 1).
  - Depthwise conv runs on the PE via diag(w_k) matmuls (off the DVE).
  - Whole scan datapath in bf16 (DVE 2x mode for the big elementwise
    ops; scan rate itself is dtype-independent). Scan state is fp32
    internally so bf16 operands only round per-step inputs.
  - One scan instruction per (block, chunk) covering all 16 states;
    section boundaries broken by zeroing a8 at section starts and
    injecting the carry into u8.
  - dt (softplus) computed for all t in a separate pass: Exp-run then
    Ln-run, so ACT table loads happen O(1) times, not per iteration.
  - a8 = exp(A_n * dt) stays on ACT (16 ops per block-chunk).
  - out_proj + residual + LayerNorm in a final phase; residual is
    injected into PSUM via identity matmul; mean/var via bn_stats;
    rsqrt batched over all tiles (DVE reciprocal + one ACT Sqrt).
"""

import numpy as np

B = 8
L = 2048
DIM = 768
DSTATE = 16
DCONV = 4
DINNER = 1536
DTRANK = 48

NCH = DINNER // 128          # 12 channel chunks
CCH = DIM // 128             # 6 dim chunks
TB = 512                     # scan time block
NBLK = L // TB
NTT = L // 512               # matmul t tiles
NTC = L // 128               # out/LN row tiles
EPS = 1e-5

_CACHE = {}


def _build():
    from contextlib import ExitStack
    import concourse.bacc as bacc
    import concourse.tile as tile
    import concourse.mybir as mybir
    from concourse.masks import make_identity

    f32 = mybir.dt.float32
    bf16 = mybir.dt.bfloat16
    AF = mybir.ActivationFunctionType
    OP = mybir.AluOpType
    AX = mybir.AxisListType

    nc = bacc.Bacc("TRN2", target_bir_lowering=False, debug=False)

    x_d = nc.dram_tensor("x", [L, DIM], f32, kind="ExternalInput").ap()
    sal_d = nc.dram_tensor("sal", [L, 1], f32, kind="ExternalInput").ap()
    spw1_d = nc.dram_tensor("sp_w1", [1, DIM // 4], f32, kind="ExternalInput").ap()
    spb1_d = nc.dram_tensor("sp_b1", [DIM // 4], f32, kind="ExternalInput").ap()
    spw2_d = nc.dram_tensor("sp_w2", [DIM // 4, 2 * DIM], f32, kind="ExternalInput").ap()
    spb2_d = nc.dram_tensor("sp_b2", [2 * DIM], f32, kind="ExternalInput").ap()
    win_d = nc.dram_tensor("in_proj_w", [DIM, 2 * DINNER], f32, kind="ExternalInput").ap()
    wcv_d = nc.dram_tensor("conv_w", [DINNER, DCONV], f32, kind="ExternalInput").ap()
    cvb_d = nc.dram_tensor("conv_b", [DINNER], f32, kind="ExternalInput").ap()
    wxp_d = nc.dram_tensor("x_proj_w", [DINNER, DTRANK + 2 * DSTATE], f32, kind="ExternalInput").ap()
    wdt_d = nc.dram_tensor("dt_proj_w", [DTRANK, DINNER], f32, kind="ExternalInput").ap()
    dtb_d = nc.dram_tensor("dt_proj_b", [DINNER], f32, kind="ExternalInput").ap()
    alog_d = nc.dram_tensor("A_log", [DINNER, DSTATE], f32, kind="ExternalInput").ap()
    dD_d = nc.dram_tensor("D", [DINNER], f32, kind="ExternalInput").ap()
    wout_d = nc.dram_tensor("out_proj_w", [DINNER, DIM], f32, kind="ExternalInput").ap()
    lng_d = nc.dram_tensor("ln_g", [DIM], f32, kind="ExternalInput").ap()
    lnb_d = nc.dram_tensor("ln_b", [DIM], f32, kind="ExternalInput").ap()
    out_d = nc.dram_tensor("out", [L, DIM], f32, kind="ExternalOutput").ap()

    xc_d = nc.dram_tensor("xc_scr", [NCH, 128, L], bf16).ap()
    zs_d = nc.dram_tensor("zs_scr", [NCH, 128, L], bf16).ap()
    dt_d = nc.dram_tensor("dt_scr", [NCH, 128, L], bf16).ap()
    bc_d = nc.dram_tensor("bc_scr", [2, DSTATE, L], bf16).ap()
    xbf_d = nc.dram_tensor("xbf_scr", [L, DIM], bf16).ap()

    with tile.TileContext(nc) as tc, ExitStack() as ctx:
        # ---------- long-lived constants ----------
        consts = ctx.enter_context(tc.tile_pool(name="consts", bufs=1))

        identb = consts.tile([128, 128], bf16, tag="identb")
        make_identity(nc, identb[:])

        A_t = []
        for i in range(NCH):
            al = consts.tile([128, DSTATE], f32, tag=f"alog{i}")
            nc.sync.dma_start(al[:], alog_d[i * 128:(i + 1) * 128, :])
            at = consts.tile([128, DSTATE], f32, tag=f"at{i}")
            nc.scalar.activation(at[:], al[:], AF.Exp)
            nc.vector.tensor_scalar_mul(at[:], at[:], -1.0)
            A_t.append(at)

        def col_per_chunk(src_vec, name):
            t = consts.tile([128, NCH], f32, tag=name)
            nc.sync.dma_start(
                t[:], src_vec.rearrange("(i p) -> i p", p=128).transpose([1, 0]))
            return t

        dtpb = col_per_chunk(dtb_d, "dtpb")
        dDc = col_per_chunk(dD_d, "dDc")
        cvbc = col_per_chunk(cvb_d, "cvbc")
        lngb = consts.tile([128, DIM], bf16, tag="lngb")
        nc.gpsimd.dma_start(lngb[:], lng_d.partition_broadcast(128))
        lnbb = consts.tile([128, DIM], f32, tag="lnbb")
        nc.sync.dma_start(lnbb[:], lnb_d.partition_broadcast(128))

        # conv diag weights: diag(w_k) per chunk, bf16
        wcvc = consts.tile([128, NCH * DCONV], f32, tag="wcvc")
        nc.sync.dma_start(
            wcvc[:], wcv_d.rearrange("(i p) k -> i p k", p=128).transpose([1, 0, 2]))
        cdiag = []
        for i in range(NCH):
            row = []
            for k in range(DCONV):
                t = consts.tile([128, 128], bf16, tag=f"cd{i}_{k}")
                nc.vector.tensor_scalar(
                    t[:], identb[:], wcvc[:, i * DCONV + k:i * DCONV + k + 1],
                    None, op0=OP.mult)
                row.append(t)
            cdiag.append(row)

        dtw = []
        for i in range(NCH):
            t = consts.tile([DTRANK, 128], bf16, tag=f"dtw{i}")
            nc.gpsimd.dma_start(t[:], wdt_d[:, i * 128:(i + 1) * 128])
            dtw.append(t)

        xpw = []
        for i in range(NCH):
            t = consts.tile([128, 112], bf16, tag=f"xpw{i}")
            nc.vector.memset(t[:], 0.0)
            isl = slice(i * 128, (i + 1) * 128)
            nc.gpsimd.dma_start(t[:, 0:DTRANK], wxp_d[isl, 0:DTRANK])
            nc.gpsimd.dma_start(t[:, 64:80], wxp_d[isl, DTRANK:DTRANK + DSTATE])
            nc.gpsimd.dma_start(t[:, 96:112], wxp_d[isl, DTRANK + DSTATE:])
            xpw.append(t)

        # out_proj weights, pre-scaled by 0.1 (residual r = x + 0.1*mamba)
        woutb = []
        with tc.tile_pool(name="wstage", bufs=2) as wstage:
            for i in range(NCH):
                wf = wstage.tile([128, DIM], f32, tag="woutf")
                nc.sync.dma_start(wf[:], wout_d[i * 128:(i + 1) * 128, :])
                t = consts.tile([128, DIM], bf16, tag=f"woutb{i}")
                nc.scalar.activation(t[:], wf[:], AF.Copy, scale=0.1)
                woutb.append(t)

        dtin_sb = consts.tile([DTRANK, L], bf16, tag="dtin")
        epsc = consts.tile([128, 1], f32, tag="epsc")
        nc.vector.memset(epsc[:], EPS)
        cys = [consts.tile([128, DSTATE], f32, tag=f"cy{i}", name=f"cy{i}")
               for i in range(NCH)]

        # ---------- phase A: FiLM + transpose ----------
        with tc.tile_pool(name="xmod", bufs=1) as xmod_pool:
            xmod = [xmod_pool.tile([128, L], bf16, tag=f"xm{cc}", name=f"xm{cc}")
                    for cc in range(CCH)]

            with tc.tile_pool(name="pa", bufs=2) as pA, \
                 tc.tile_pool(name="pa_c", bufs=1) as pAc, \
                 tc.tile_pool(name="pa_ps", bufs=2, space="PSUM") as pA_ps:
                ones96 = pAc.tile([1, 96], bf16, tag="ones96")
                nc.vector.memset(ones96[:], 1.0)
                w1c = pAc.tile([96, 2], f32, tag="w1c")
                nc.sync.dma_start(
                    w1c[:], spw1_d.rearrange("o (g j) -> o g j", g=2).squeeze(0).transpose([1, 0]))
                b1c = pAc.tile([96, 2], f32, tag="b1c")
                nc.sync.dma_start(b1c[:], spb1_d.rearrange("(g j) -> g j", g=2).transpose([1, 0]))
                spb2c = pAc.tile([128, 12], f32, tag="spb2")
                nc.sync.dma_start(
                    spb2c[:], spb2_d.rearrange("(i p) -> i p", p=128).transpose([1, 0]))
                w2c = []
                for kc in range(2):
                    row = []
                    for m in range(12):
                        t = pAc.tile([96, 128], bf16, tag=f"w2c{kc}_{m}")
                        nc.gpsimd.dma_start(
                            t[:], spw2_d[kc * 96:(kc + 1) * 96, m * 128:(m + 1) * 128])
                        row.append(t)
                    w2c.append(row)

                # saliency broadcast + FiLM hidden layer
                sal_sb = pAc.tile([1, L], bf16, tag="salsb")
                nc.gpsimd.dma_start(sal_sb[:], sal_d.transpose([1, 0]))
                h2 = [pAc.tile([96, L], bf16, tag=f"h2_{kc}", name=f"h2_{kc}")
                      for kc in range(2)]
                for kc in range(2):
                    for tt in range(NTT):
                        ps = pA_ps.tile([96, 512], f32, tag="salps")
                        nc.tensor.matmul(ps[:], ones96[:],
                                         sal_sb[:, tt * 512:(tt + 1) * 512],
                                         start=True, stop=True)
                        nc.scalar.activation(h2[kc][:, tt * 512:(tt + 1) * 512], ps[:],
                                             AF.Relu, scale=w1c[:, kc:kc + 1],
                                             bias=b1c[:, kc:kc + 1])

                # x transpose -> xmod tiles hold xT (bf16) via DMA xbar:
                # cast x to bf16 in DRAM, then transpose-DMA column blocks.
                nc.gpsimd.dma_start(xbf_d, x_d)
                for cc in range(CCH):
                    nc.sync.dma_start_transpose(
                        xmod[cc][:], xbf_d[:, cc * 128:(cc + 1) * 128])

                # FiLM affine + modulation: x_mod = x*tanh(g) + (x + beta)
                for cc in range(CCH):
                    for tt in range(NTT):
                        sl = slice(tt * 512, (tt + 1) * 512)
                        psg = pA_ps.tile([128, 512], f32, tag="affg")
                        for kc in range(2):
                            nc.tensor.matmul(psg[:], w2c[kc][cc][:], h2[kc][:, sl],
                                             start=(kc == 0), stop=(kc == 1))
                        tg = pA.tile([128, 512], bf16, tag="tg")
                        nc.scalar.activation(tg[:], psg[:], AF.Tanh,
                                             bias=spb2c[:, cc:cc + 1])
                        psb = pA_ps.tile([128, 512], f32, tag="affb")
                        for kc in range(2):
                            nc.tensor.matmul(psb[:], w2c[kc][cc + 6][:], h2[kc][:, sl],
                                             start=(kc == 0), stop=False)
                        # inject x so bt = x + beta
                        nc.tensor.matmul(psb[:], identb[:], xmod[cc][:, sl],
                                         start=False, stop=True)
                        bt = pA.tile([128, 512], bf16, tag="bt")
                        nc.scalar.activation(bt[:], psb[:], AF.Identity,
                                             bias=spb2c[:, cc + 6:cc + 7])
                        t1 = pA.tile([128, 512], bf16, tag="t1")
                        nc.vector.tensor_tensor(t1[:], xmod[cc][:, sl], tg[:], OP.mult)
                        nc.vector.tensor_tensor(xmod[cc][:, sl], t1[:], bt[:], OP.add)

            # ---------- phase B1: in_proj -> xin tiles + silu(z) ----------
            with tc.tile_pool(name="xin_pool", bufs=1) as xin_pool:
                xins = [xin_pool.tile([128, L + 3], bf16, tag=f"xin{i}", name=f"xin{i}")
                        for i in range(NCH)]
                with tc.tile_pool(name="pb", bufs=2) as pB, \
                     tc.tile_pool(name="pb_w", bufs=3) as pB_w, \
                     tc.tile_pool(name="pb_ps", bufs=2, space="PSUM") as pB_ps:
                    for m in range(24):
                        psl = pB_ps.tile([128, L], f32, tag="ipp", name=f"ipp{m}")
                        for cc in range(CCH):
                            wt = pB_w.tile([128, 128], bf16, tag="wstage")
                            nc.gpsimd.dma_start(
                                wt[:], win_d[cc * 128:(cc + 1) * 128, m * 128:(m + 1) * 128])
                            for tt in range(NTT):
                                nc.tensor.matmul(psl[:, tt * 512:(tt + 1) * 512],
                                                 wt[:],
                                                 xmod[cc][:, tt * 512:(tt + 1) * 512],
                                                 start=(cc == 0), stop=(cc == CCH - 1))
                        if m >= 12:
                            i = m - 12
                            zt = pB.tile([128, L], bf16, tag="ztile")
                            for tt in range(NTT):
                                nc.scalar.activation(zt[:, tt * 512:(tt + 1) * 512],
                                                     psl[:, tt * 512:(tt + 1) * 512],
                                                     AF.Silu)
                            nc.sync.dma_start(zs_d[i], zt[:])
                        else:
                            i = m
                            nc.vector.memset(xins[i][:, 0:3], 0.0)
                            for tt in range(NTT):
                                nc.scalar.copy(
                                    xins[i][:, 3 + tt * 512:3 + (tt + 1) * 512],
                                    psl[:, tt * 512:(tt + 1) * 512])

                # ---------- phase B2: conv on PE + silu ----------
                with tc.tile_pool(name="pc2", bufs=2) as pC2, \
                     tc.tile_pool(name="pc2_ps", bufs=4, space="PSUM") as pC2_ps:
                    for i in range(NCH):
                        xct = pC2.tile([128, L], bf16, tag="xct")
                        for tt in range(NTT):
                            cps = pC2_ps.tile([128, 512], f32, tag="cps")
                            for k in range(DCONV):
                                nc.tensor.matmul(
                                    cps[:], cdiag[i][k][:],
                                    xins[i][:, k + tt * 512:k + (tt + 1) * 512],
                                    start=(k == 0), stop=(k == DCONV - 1))
                            nc.scalar.activation(xct[:, tt * 512:(tt + 1) * 512],
                                                 cps[:], AF.Silu,
                                                 bias=cvbc[:, i:i + 1])
                        nc.sync.dma_start(xc_d[i], xct[:])

        # ---------- phase C: x_proj ----------
        with tc.tile_pool(name="pc", bufs=2) as pC, \
             tc.tile_pool(name="pc_ps", bufs=1, space="PSUM") as pC_ps:
            psd = pC_ps.tile([112, L], f32, tag="psd")
            for i in range(NCH):
                xci = pC.tile([128, L], bf16, tag="xcld")
                nc.sync.dma_start(xci[:], xc_d[i])
                for tt in range(NTT):
                    nc.tensor.matmul(psd[:, tt * 512:(tt + 1) * 512], xpw[i][:],
                                     xci[:, tt * 512:(tt + 1) * 512],
                                     start=(i == 0), stop=(i == NCH - 1))
            nc.scalar.copy(dtin_sb[:], psd[0:DTRANK, :])
            bcb = pC.tile([DSTATE, L], bf16, tag="bcb")
            nc.scalar.copy(bcb[:], psd[64:80, :])
            nc.sync.dma_start(bc_d[0], bcb[:])
            bcc = pC.tile([DSTATE, L], bf16, tag="bcc")
            nc.scalar.copy(bcc[:], psd[96:112, :])
            nc.sync.dma_start(bc_d[1], bcc[:])

        # ---------- phase C2: dt = softplus(dtin @ dt_proj_w + b) ----------
        with tc.tile_pool(name="pdt", bufs=2) as pDT, \
             tc.tile_pool(name="pdt_ps", bufs=2, space="PSUM") as pDT_ps:
            evs = []
            for i in range(NCH):
                ps = pDT_ps.tile([128, L], f32, tag="dtps", name=f"dtps{i}")
                for tt in range(NTT):
                    nc.tensor.matmul(ps[:, tt * 512:(tt + 1) * 512], dtw[i][:],
                                     dtin_sb[:, tt * 512:(tt + 1) * 512],
                                     start=True, stop=True)
                ev = pDT.tile([128, L], bf16, tag=f"ev{i % 4}", name=f"ev{i}")
                nc.scalar.activation(ev[:], ps[:], AF.Exp, bias=dtpb[:, i:i + 1])
                evs.append(ev)
                if i >= 3:
                    j = i - 3
                    dtt = pDT.tile([128, L], bf16, tag="dtt")
                    nc.scalar.activation(dtt[:], evs[j][:], AF.Ln, bias=1.0)
                    nc.sync.dma_start(dt_d[j], dtt[:])
            for j in range(NCH - 3, NCH):
                dtt = pDT.tile([128, L], bf16, tag="dtt")
                nc.scalar.activation(dtt[:], evs[j][:], AF.Ln, bias=1.0)
                nc.sync.dma_start(dt_d[j], dtt[:])

        # ---------- phase D: selective scan + per-block out_proj/LN ----------
        with tc.tile_pool(name="pbc", bufs=1) as pBC, \
             tc.tile_pool(name="pd_s", bufs=2) as pS, \
             tc.tile_pool(name="pd_w", bufs=1) as pW, \
             tc.tile_pool(name="pyg", bufs=2) as pYG, \
             tc.tile_pool(name="pe", bufs=2) as pE, \
             tc.tile_pool(name="pe_r", bufs=1) as pER, \
             tc.tile_pool(name="pe_ps", bufs=2, space="PSUM") as pE_ps:
            for blk in range(NBLK):
                tsl = slice(blk * TB, (blk + 1) * TB)
                Bb = pBC.tile([128, DSTATE * TB], bf16, tag="Bb", name=f"Bb{blk}")
                Cb = pBC.tile([128, DSTATE * TB], bf16, tag="Cb", name=f"Cb{blk}")
                nc.gpsimd.dma_start(Bb[:], bc_d[0, :, tsl].partition_broadcast(128))
                nc.gpsimd.dma_start(Cb[:], bc_d[1, :, tsl].partition_broadcast(128))

                ygs = [pYG.tile([128, TB], bf16, tag=f"yg{i}", name=f"yg{blk}_{i}")
                       for i in range(NCH)]
                for i in range(NCH):
                    dt_t = pS.tile([128, TB], bf16, tag="dtt")
                    nc.sync.dma_start(dt_t[:], dt_d[i, :, tsl])
                    xc_t = pS.tile([128, TB], bf16, tag="xctd")
                    nc.sync.dma_start(xc_t[:], xc_d[i, :, tsl])
                    zs_t = pS.tile([128, TB], bf16, tag="zstd")
                    nc.sync.dma_start(zs_t[:], zs_d[i, :, tsl])

                    # a8[n] = exp(A[:,n] * dt)  (16 ACT ops; double-buffered
                    # so ACT runs one iteration ahead of the DVE scan)
                    a8 = pW.tile([128, DSTATE * TB], bf16, tag="a8", bufs=2)
                    for n in range(DSTATE):
                        nc.scalar.activation(a8[:, n * TB:(n + 1) * TB], dt_t[:],
                                             AF.Exp, scale=A_t[i][:, n:n + 1])

                    dtx = pS.tile([128, TB], bf16, tag="dtx")
                    nc.vector.tensor_tensor(dtx[:], dt_t[:], xc_t[:], OP.mult)
                    u8 = pW.tile([128, DSTATE * TB], bf16, tag="u8")
                    nc.vector.tensor_tensor(
                        u8[:], dtx[:][:, None, :].broadcast_to([128, DSTATE, TB]),
                        Bb[:].rearrange("p (s t) -> p s t", s=DSTATE), OP.mult)

                    a8v = a8[:].rearrange("p (s t) -> p s t", s=DSTATE)
                    u8v = u8[:].rearrange("p (s t) -> p s t", s=DSTATE)
                    if blk > 0:
                        tmp = pS.tile([128, DSTATE], f32, tag="cytmp")
                        nc.vector.tensor_tensor(
                            tmp[:], a8v[:, :, 0:1].squeeze(), cys[i][:], OP.mult)
                        nc.vector.tensor_tensor(
                            u8v[:, :, 0:1].squeeze(),
                            u8v[:, :, 0:1].squeeze(), tmp[:], OP.add)
                    nc.vector.memset(a8v[:, :, 0:1], 0.0)

                    h8 = pW.tile([128, DSTATE * TB], bf16, tag="h8")
                    nc.vector.tensor_tensor_scan(h8[:], a8[:], u8[:], 0.0,
                                                 OP.mult, OP.add)
                    if blk < NBLK - 1:
                        nc.vector.tensor_copy(
                            cys[i][:],
                            h8[:].rearrange("p (s t) -> p s t",
                                            s=DSTATE)[:, :, TB - 1:TB].squeeze())

                    # y = sum_n C_n * h_n  (mult + pairwise tree, all bf16);
                    # u8 is dead after the scan, reuse its buffer for prod
                    prod = pW.tile([128, DSTATE * TB], bf16, tag="u8",
                                   name=f"prod{blk}_{i}")
                    nc.vector.tensor_tensor(prod[:], h8[:], Cb[:], OP.mult)
                    nc.vector.tensor_tensor(prod[:, 0:8 * TB], prod[:, 0:8 * TB],
                                            prod[:, 8 * TB:16 * TB], OP.add)
                    nc.vector.tensor_tensor(prod[:, 0:4 * TB], prod[:, 0:4 * TB],
                                            prod[:, 4 * TB:8 * TB], OP.add)
                    nc.vector.tensor_tensor(prod[:, 0:2 * TB], prod[:, 0:2 * TB],
                                            prod[:, 2 * TB:4 * TB], OP.add)
                    y1 = pS.tile([128, TB], bf16, tag="y1")
                    nc.vector.tensor_tensor(y1[:], prod[:, 0:TB],
                                            prod[:, TB:2 * TB], OP.add)
                    # y += D*xc ; gate with silu(z)
                    y2 = pS.tile([128, TB], bf16, tag="y2")
                    nc.vector.scalar_tensor_tensor(
                        y2[:], xc_t[:], dDc[:, i:i + 1], y1[:],
                        op0=OP.mult, op1=OP.add)
                    nc.vector.tensor_tensor(ygs[i][:], y2[:], zs_t[:], OP.mult)

                # ---- out_proj + residual + LayerNorm for this block ----
                rts = [pER.tile([128, DIM], bf16, tag=f"rt{t4}",
                                name=f"rt{blk}_{t4}") for t4 in range(TB // 128)]
                mv = pER.tile([128, 2 * (TB // 128)], f32, tag="mv",
                              name=f"mv{blk}")
                for t4 in range(TB // 128):
                    trow = blk * TB + t4 * 128
                    xres = pE.tile([128, DIM], bf16, tag="xres")
                    nc.sync.dma_start(xres[:], xbf_d[trow:trow + 128, :])
                    ps1 = pE_ps.tile([128, 512], f32, tag="op1")
                    ps2 = pE_ps.tile([128, 256], f32, tag="op2")
                    nc.tensor.matmul(ps1[:], identb[:], xres[:, 0:512],
                                     start=True, stop=False)
                    nc.tensor.matmul(ps2[:], identb[:], xres[:, 512:768],
                                     start=True, stop=False)
                    for i in range(NCH):
                        lhs = ygs[i][:, t4 * 128:(t4 + 1) * 128]
                        nc.tensor.matmul(ps1[:], lhs, woutb[i][:, 0:512],
                                         start=False, stop=(i == NCH - 1))
                        nc.tensor.matmul(ps2[:], lhs, woutb[i][:, 512:768],
                                         start=False, stop=(i == NCH - 1))
                    nc.scalar.copy(rts[t4][:, 0:512], ps1[:])
                    nc.scalar.copy(rts[t4][:, 512:768], ps2[:])
                    st6 = pE.tile([128, 12], f32, tag="st6")
                    nc.vector.bn_stats(st6[:, 0:6], rts[t4][:, 0:512])
                    nc.vector.bn_stats(st6[:, 6:12], rts[t4][:, 512:768])
                    nc.vector.bn_aggr(mv[:, 2 * t4:2 * t4 + 2],
                                      st6[:].rearrange("p (g s) -> p g s", g=2))

                # rstd = exp(-0.5*ln(var+eps)), batched over the block
                vare = pE.tile([128, TB // 128], f32, tag="vare")
                nc.vector.tensor_scalar(
                    vare[:], mv[:].rearrange("p (t s) -> p t s", s=2)[:, :, 1:2].squeeze(),
                    epsc[:], None, op0=OP.add)
                lnv = pE.tile([128, TB // 128], f32, tag="lnv")
                nc.scalar.activation(lnv[:], vare[:], AF.Ln)
                rstd = pE.tile([128, TB // 128], f32, tag="rstd", bufs=2)
                nc.scalar.activation(rstd[:], lnv[:], AF.Exp, scale=-0.5)

                for t4 in range(TB // 128):
                    trow = blk * TB + t4 * 128
                    t1 = pE.tile([128, DIM], bf16, tag="t1")
                    nc.vector.tensor_scalar(
                        t1[:], rts[t4][:], mv[:, 2 * t4:2 * t4 + 1],
                        rstd[:, t4:t4 + 1], op0=OP.subtract, op1=OP.mult)
                    t2 = pE.tile([128, DIM], bf16, tag="t2")
                    nc.vector.tensor_tensor(t2[:], t1[:], lngb[:], OP.mult)
                    of = pE.tile([128, DIM], f32, tag="of")
                    nc.vector.tensor_tensor(of[:], t2[:], lnbb[:], OP.add)
                    nc.sync.dma_start(out_d[trow:trow + 128, :], of[:])

    nc.compile()
    return nc


def _get_nc():
    if "nc" not in _CACHE:
        _CACHE["nc"] = _build()
    return _CACHE["nc"]


def kernel(**inputs):
    from concourse.bass_utils import run_bass_kernel_spmd

    nc = _get_nc()
    shared = {k: np.ascontiguousarray(np.asarray(inputs[k], np.float32))
              for k in ("sp_w1", "sp_b1", "sp_w2", "sp_b2", "in_proj_w", "conv_w",
                        "conv_b", "x_proj_w", "dt_proj_w", "dt_proj_b", "A_log",
                        "D", "out_proj_w", "ln_g", "ln_b")}
    x = np.asarray(inputs["x"], np.float32)
    sal = np.asarray(inputs["saliency_score"], np.float32)
    in_maps = []
    for c in range(B):
        m = dict(shared)
        m["x"] = np.ascontiguousarray(x[c])
        m["sal"] = np.ascontiguousarray(sal[c])
        in_maps.append(m)
    res = run_bass_kernel_spmd(nc, in_maps, core_ids=list(range(B)))
    out = np.stack([res.results[c]["out"] for c in range(B)], axis=0)
    return out


# revision 25
# speedup vs baseline: 1.4967x; 1.0509x over previous
"""ContentAwareMambaFilter Trainium2 kernel (v2, bf16 datapath).

Data-parallel over batch: 8 NeuronCores, one batch row each. Takes full
(unsharded) inputs, returns the full output; per-core slicing happens in
kernel(). The Bass program is built and compiled once, then cached.

Key design points vs v1:
  - All matmuls in bf16 (fp32 PE is 4 cycles/row, bf16 is 1).
  - Depthwise conv runs on the PE via diag(w_k) matmuls (off the DVE).
  - Whole scan datapath in bf16 (DVE 2x mode for the big elementwise
    ops; scan rate itself is dtype-independent). Scan state is fp32
    internally so bf16 operands only round per-step inputs.
  - One scan instruction per (block, chunk) covering all 16 states;
    section boundaries broken by zeroing a8 at section starts and
    injecting the carry into u8.
  - dt (softplus) computed for all t in a separate pass: Exp-run then
    Ln-run, so ACT table loads happen O(1) times, not per iteration.
  - a8 = exp(A_n * dt) stays on ACT (16 ops per block-chunk).
  - out_proj + residual + LayerNorm in a final phase; residual is
    injected into PSUM via identity matmul; mean/var via bn_stats;
    rsqrt batched over all tiles (DVE reciprocal + one ACT Sqrt).
"""

import numpy as np

B = 8
L = 2048
DIM = 768
DSTATE = 16
DCONV = 4
DINNER = 1536
DTRANK = 48

NCH = DINNER // 128          # 12 channel chunks
CCH = DIM // 128             # 6 dim chunks
TB = 512                     # scan time block
NBLK = L // TB
NTT = L // 512               # matmul t tiles
NTC = L // 128               # out/LN row tiles
EPS = 1e-5

_CACHE = {}


def _build():
    from contextlib import ExitStack
    import concourse.bacc as bacc
    import concourse.tile as tile
    import concourse.mybir as mybir
    from concourse.masks import make_identity

    f32 = mybir.dt.float32
    bf16 = mybir.dt.bfloat16
    AF = mybir.ActivationFunctionType
    OP = mybir.AluOpType
    AX = mybir.AxisListType

    nc = bacc.Bacc("TRN2", target_bir_lowering=False, debug=False)

    x_d = nc.dram_tensor("x", [L, DIM], f32, kind="ExternalInput").ap()
    sal_d = nc.dram_tensor("sal", [L, 1], f32, kind="ExternalInput").ap()
    spw1_d = nc.dram_tensor("sp_w1", [1, DIM // 4], f32, kind="ExternalInput").ap()
    spb1_d = nc.dram_tensor("sp_b1", [DIM // 4], f32, kind="ExternalInput").ap()
    spw2_d = nc.dram_tensor("sp_w2", [DIM // 4, 2 * DIM], f32, kind="ExternalInput").ap()
    spb2_d = nc.dram_tensor("sp_b2", [2 * DIM], f32, kind="ExternalInput").ap()
    win_d = nc.dram_tensor("in_proj_w", [DIM, 2 * DINNER], f32, kind="ExternalInput").ap()
    wcv_d = nc.dram_tensor("conv_w", [DINNER, DCONV], f32, kind="ExternalInput").ap()
    cvb_d = nc.dram_tensor("conv_b", [DINNER], f32, kind="ExternalInput").ap()
    wxp_d = nc.dram_tensor("x_proj_w", [DINNER, DTRANK + 2 * DSTATE], f32, kind="ExternalInput").ap()
    wdt_d = nc.dram_tensor("dt_proj_w", [DTRANK, DINNER], f32, kind="ExternalInput").ap()
    dtb_d = nc.dram_tensor("dt_proj_b", [DINNER], f32, kind="ExternalInput").ap()
    alog_d = nc.dram_tensor("A_log", [DINNER, DSTATE], f32, kind="ExternalInput").ap()
    dD_d = nc.dram_tensor("D", [DINNER], f32, kind="ExternalInput").ap()
    wout_d = nc.dram_tensor("out_proj_w", [DINNER, DIM], f32, kind="ExternalInput").ap()
    lng_d = nc.dram_tensor("ln_g", [DIM], f32, kind="ExternalInput").ap()
    lnb_d = nc.dram_tensor("ln_b", [DIM], f32, kind="ExternalInput").ap()
    out_d = nc.dram_tensor("out", [L, DIM], f32, kind="ExternalOutput").ap()

    xc_d = nc.dram_tensor("xc_scr", [NCH, 128, L], bf16).ap()
    zs_d = nc.dram_tensor("zs_scr", [NCH, 128, L], bf16).ap()
    dt_d = nc.dram_tensor("dt_scr", [NCH, 128, L], bf16).ap()
    bc_d = nc.dram_tensor("bc_scr", [2, DSTATE, L], bf16).ap()
    xbf_d = nc.dram_tensor("xbf_scr", [L, DIM], bf16).ap()

    with tile.TileContext(nc) as tc, ExitStack() as ctx:
        # ---------- long-lived constants ----------
        consts = ctx.enter_context(tc.tile_pool(name="consts", bufs=1))

        identb = consts.tile([128, 128], bf16, tag="identb")
        make_identity(nc, identb[:])

        # A = -exp(A_log), all chunks in one load/exp/neg
        al_all = consts.tile([128, NCH * DSTATE], f32, tag="alall")
        nc.sync.dma_start(
            al_all[:].rearrange("p (i n) -> p i n", n=DSTATE),
            alog_d.rearrange("(i p) n -> p i n", p=128))
        at_all = consts.tile([128, NCH * DSTATE], f32, tag="atall")
        nc.scalar.activation(at_all[:], al_all[:], AF.Exp)
        nc.vector.tensor_scalar_mul(at_all[:], at_all[:], -1.0)

        def col_per_chunk(src_vec, name):
            t = consts.tile([128, NCH], f32, tag=name)
            nc.sync.dma_start(
                t[:], src_vec.rearrange("(i p) -> i p", p=128).transpose([1, 0]))
            return t

        dtpb = col_per_chunk(dtb_d, "dtpb")
        dDc = col_per_chunk(dD_d, "dDc")
        cvbc = col_per_chunk(cvb_d, "cvbc")
        lngb = consts.tile([128, DIM], bf16, tag="lngb")
        nc.gpsimd.dma_start(lngb[:], lng_d.partition_broadcast(128))
        lnbb = consts.tile([128, DIM], f32, tag="lnbb")
        nc.sync.dma_start(lnbb[:], lnb_d.partition_broadcast(128))

        # conv diag weights: diag(w_k) per chunk, bf16
        wcvc = consts.tile([128, NCH * DCONV], f32, tag="wcvc")
        nc.sync.dma_start(
            wcvc[:], wcv_d.rearrange("(i p) k -> i p k", p=128).transpose([1, 0, 2]))
        cdiag = []
        for i in range(NCH):
            row = []
            for k in range(DCONV):
                t = consts.tile([128, 128], bf16, tag=f"cd{i}_{k}")
                nc.vector.tensor_scalar(
                    t[:], identb[:], wcvc[:, i * DCONV + k:i * DCONV + k + 1],
                    None, op0=OP.mult)
                row.append(t)
            cdiag.append(row)

        # dt_proj weights: one cast DMA for all chunks
        dtw_all = consts.tile([DTRANK, DINNER], bf16, tag="dtwall")
        nc.gpsimd.dma_start(dtw_all[:], wdt_d)
        dtw = [dtw_all[:, i * 128:(i + 1) * 128] for i in range(NCH)]

        # x_proj weights: padded layout (dt 0:48, B 64:80, C 96:112) for all
        # chunks, filled with 3 strided cast DMAs
        xpw_all = consts.tile([128, NCH * 112], bf16, tag="xpwall")
        nc.vector.memset(xpw_all[:], 0.0)
        xpv = xpw_all[:].rearrange("p (i c) -> p i c", c=112)
        wxv = wxp_d.rearrange("(i p) c -> p i c", p=128)
        nc.gpsimd.dma_start(xpv[:, :, 0:DTRANK], wxv[:, :, 0:DTRANK])
        nc.gpsimd.dma_start(xpv[:, :, 64:80], wxv[:, :, DTRANK:DTRANK + DSTATE])
        nc.gpsimd.dma_start(xpv[:, :, 96:112], wxv[:, :, DTRANK + DSTATE:])
        xpw = [xpw_all[:, i * 112:(i + 1) * 112] for i in range(NCH)]

        # out_proj weights, pre-scaled by 0.1 (residual r = x + 0.1*mamba)
        woutb = []
        with tc.tile_pool(name="wstage", bufs=2) as wstage:
            for i in range(NCH):
                wf = wstage.tile([128, DIM], f32, tag="woutf")
                nc.sync.dma_start(wf[:], wout_d[i * 128:(i + 1) * 128, :])
                t = consts.tile([128, DIM], bf16, tag=f"woutb{i}")
                nc.scalar.activation(t[:], wf[:], AF.Copy, scale=0.1)
                woutb.append(t)

        dtin_sb = consts.tile([DTRANK, L], bf16, tag="dtin")
        epsc = consts.tile([128, 1], f32, tag="epsc")
        nc.vector.memset(epsc[:], EPS)
        cys = [consts.tile([128, DSTATE], f32, tag=f"cy{i}", name=f"cy{i}")
               for i in range(NCH)]

        # ---------- phase A: FiLM + transpose ----------
        with tc.tile_pool(name="xmod", bufs=1) as xmod_pool:
            xmod = [xmod_pool.tile([128, L], bf16, tag=f"xm{cc}", name=f"xm{cc}")
                    for cc in range(CCH)]

            with tc.tile_pool(name="pa", bufs=2) as pA, \
                 tc.tile_pool(name="pa_c", bufs=1) as pAc, \
                 tc.tile_pool(name="pa_ps", bufs=2, space="PSUM") as pA_ps:
                ones96 = pAc.tile([1, 96], bf16, tag="ones96")
                nc.vector.memset(ones96[:], 1.0)
                w1c = pAc.tile([96, 2], f32, tag="w1c")
                nc.sync.dma_start(
                    w1c[:], spw1_d.rearrange("o (g j) -> o g j", g=2).squeeze(0).transpose([1, 0]))
                b1c = pAc.tile([96, 2], f32, tag="b1c")
                nc.sync.dma_start(b1c[:], spb1_d.rearrange("(g j) -> g j", g=2).transpose([1, 0]))
                spb2c = pAc.tile([128, 12], f32, tag="spb2")
                nc.sync.dma_start(
                    spb2c[:], spb2_d.rearrange("(i p) -> i p", p=128).transpose([1, 0]))
                w2c_all = pAc.tile([96, 2 * 2 * DIM], bf16, tag="w2call")
                nc.gpsimd.dma_start(
                    w2c_all[:].rearrange("o (kc d) -> o kc d", kc=2),
                    spw2_d.rearrange("(kc o) d -> o kc d", kc=2))
                w2c = [[w2c_all[:, kc * 2 * DIM + m * 128:kc * 2 * DIM + (m + 1) * 128]
                        for m in range(12)] for kc in range(2)]

                # saliency broadcast + FiLM hidden layer
                sal_sb = pAc.tile([1, L], bf16, tag="salsb")
                nc.gpsimd.dma_start(sal_sb[:], sal_d.transpose([1, 0]))
                h2 = [pAc.tile([96, L], bf16, tag=f"h2_{kc}", name=f"h2_{kc}")
                      for kc in range(2)]
                for kc in range(2):
                    for tt in range(NTT):
                        ps = pA_ps.tile([96, 512], f32, tag="salps")
                        nc.tensor.matmul(ps[:], ones96[:],
                                         sal_sb[:, tt * 512:(tt + 1) * 512],
                                         start=True, stop=True)
                        nc.scalar.activation(h2[kc][:, tt * 512:(tt + 1) * 512], ps[:],
                                             AF.Relu, scale=w1c[:, kc:kc + 1],
                                             bias=b1c[:, kc:kc + 1])

                # x transpose -> xmod tiles hold xT (bf16) via DMA xbar:
                # cast x to bf16 in DRAM, then transpose-DMA column blocks.
                nc.gpsimd.dma_start(xbf_d, x_d)
                for cc in range(CCH):
                    nc.sync.dma_start_transpose(
                        xmod[cc][:], xbf_d[:, cc * 128:(cc + 1) * 128])

                # FiLM affine + modulation: x_mod = x*tanh(g) + (x + beta)
                for cc in range(CCH):
                    for tt in range(NTT):
                        sl = slice(tt * 512, (tt + 1) * 512)
                        psg = pA_ps.tile([128, 512], f32, tag="affg")
                        for kc in range(2):
                            nc.tensor.matmul(psg[:], w2c[kc][cc][:], h2[kc][:, sl],
                                             start=(kc == 0), stop=(kc == 1))
                        tg = pA.tile([128, 512], bf16, tag="tg")
                        nc.scalar.activation(tg[:], psg[:], AF.Tanh,
                                             bias=spb2c[:, cc:cc + 1])
                        psb = pA_ps.tile([128, 512], f32, tag="affb")
                        for kc in range(2):
                            nc.tensor.matmul(psb[:], w2c[kc][cc + 6][:], h2[kc][:, sl],
                                             start=(kc == 0), stop=False)
                        # inject x so bt = x + beta
                        nc.tensor.matmul(psb[:], identb[:], xmod[cc][:, sl],
                                         start=False, stop=True)
                        bt = pA.tile([128, 512], bf16, tag="bt")
                        nc.scalar.activation(bt[:], psb[:], AF.Identity,
                                             bias=spb2c[:, cc + 6:cc + 7])
                        t1 = pA.tile([128, 512], bf16, tag="t1")
                        nc.vector.tensor_tensor(t1[:], xmod[cc][:, sl], tg[:], OP.mult)
                        nc.vector.tensor_tensor(xmod[cc][:, sl], t1[:], bt[:], OP.add)

            # ---------- phase B1: in_proj -> xin tiles + silu(z) ----------
            with tc.tile_pool(name="xin_pool", bufs=1) as xin_pool:
                xins = [xin_pool.tile([128, L + 3], bf16, tag=f"xin{i}", name=f"xin{i}")
                        for i in range(NCH)]
                with tc.tile_pool(name="pb", bufs=2) as pB, \
                     tc.tile_pool(name="pb_w", bufs=1) as pB_w, \
                     tc.tile_pool(name="pb_ps", bufs=2, space="PSUM") as pB_ps:
                    # preload all in_proj weights (one cast DMA per cc chunk)
                    win_all = pB_w.tile([128, CCH * 2 * DINNER], bf16,
                                        tag="winall")
                    for cc in range(CCH):
                        nc.gpsimd.dma_start(
                            win_all[:, cc * 2 * DINNER:(cc + 1) * 2 * DINNER],
                            win_d[cc * 128:(cc + 1) * 128, :])
                    for m in range(24):
                        psl = pB_ps.tile([128, L], f32, tag="ipp", name=f"ipp{m}")
                        for cc in range(CCH):
                            wt = win_all[:, cc * 2 * DINNER + m * 128:
                                         cc * 2 * DINNER + (m + 1) * 128]
                            for tt in range(NTT):
                                nc.tensor.matmul(psl[:, tt * 512:(tt + 1) * 512],
                                                 wt,
                                                 xmod[cc][:, tt * 512:(tt + 1) * 512],
                                                 start=(cc == 0), stop=(cc == CCH - 1))
                        if m >= 12:
                            i = m - 12
                            zt = pB.tile([128, L], bf16, tag="ztile")
                            for tt in range(NTT):
                                nc.scalar.activation(zt[:, tt * 512:(tt + 1) * 512],
                                                     psl[:, tt * 512:(tt + 1) * 512],
                                                     AF.Silu)
                            nc.sync.dma_start(zs_d[i], zt[:])
                        else:
                            i = m
                            nc.vector.memset(xins[i][:, 0:3], 0.0)
                            for tt in range(NTT):
                                nc.scalar.copy(
                                    xins[i][:, 3 + tt * 512:3 + (tt + 1) * 512],
                                    psl[:, tt * 512:(tt + 1) * 512])

                # ---------- phase B2: conv on PE + silu ----------
                with tc.tile_pool(name="pc2", bufs=2) as pC2, \
                     tc.tile_pool(name="pc2_ps", bufs=4, space="PSUM") as pC2_ps:
                    for i in range(NCH):
                        xct = pC2.tile([128, L], bf16, tag="xct")
                        for tt in range(NTT):
                            cps = pC2_ps.tile([128, 512], f32, tag="cps")
                            for k in range(DCONV):
                                nc.tensor.matmul(
                                    cps[:], cdiag[i][k][:],
                                    xins[i][:, k + tt * 512:k + (tt + 1) * 512],
                                    start=(k == 0), stop=(k == DCONV - 1))
                            nc.scalar.activation(xct[:, tt * 512:(tt + 1) * 512],
                                                 cps[:], AF.Silu,
                                                 bias=cvbc[:, i:i + 1])
                        nc.sync.dma_start(xc_d[i], xct[:])

        # ---------- phase C: x_proj ----------
        with tc.tile_pool(name="pc", bufs=2) as pC, \
             tc.tile_pool(name="pc_ps", bufs=1, space="PSUM") as pC_ps:
            psd = pC_ps.tile([112, L], f32, tag="psd")
            for i in range(NCH):
                xci = pC.tile([128, L], bf16, tag="xcld")
                nc.sync.dma_start(xci[:], xc_d[i])
                for tt in range(NTT):
                    nc.tensor.matmul(psd[:, tt * 512:(tt + 1) * 512], xpw[i][:],
                                     xci[:, tt * 512:(tt + 1) * 512],
                                     start=(i == 0), stop=(i == NCH - 1))
            nc.scalar.copy(dtin_sb[:], psd[0:DTRANK, :])
            bcb = pC.tile([DSTATE, L], bf16, tag="bcb")
            nc.scalar.copy(bcb[:], psd[64:80, :])
            nc.sync.dma_start(bc_d[0], bcb[:])
            bcc = pC.tile([DSTATE, L], bf16, tag="bcc")
            nc.scalar.copy(bcc[:], psd[96:112, :])
            nc.sync.dma_start(bc_d[1], bcc[:])

        # ---------- phase C2: dt = softplus(dtin @ dt_proj_w + b) ----------
        with tc.tile_pool(name="pdt", bufs=2) as pDT, \
             tc.tile_pool(name="pdt_ps", bufs=2, space="PSUM") as pDT_ps:
            evs = []
            for i in range(NCH):
                ps = pDT_ps.tile([128, L], f32, tag="dtps", name=f"dtps{i}")
                for tt in range(NTT):
                    nc.tensor.matmul(ps[:, tt * 512:(tt + 1) * 512], dtw[i][:],
                                     dtin_sb[:, tt * 512:(tt + 1) * 512],
                                     start=True, stop=True)
                ev = pDT.tile([128, L], bf16, tag=f"ev{i % 4}", name=f"ev{i}")
                nc.scalar.activation(ev[:], ps[:], AF.Exp, bias=dtpb[:, i:i + 1])
                evs.append(ev)
                if i >= 3:
                    j = i - 3
                    dtt = pDT.tile([128, L], bf16, tag="dtt")
                    nc.scalar.activation(dtt[:], evs[j][:], AF.Ln, bias=1.0)
                    nc.sync.dma_start(dt_d[j], dtt[:])
            for j in range(NCH - 3, NCH):
                dtt = pDT.tile([128, L], bf16, tag="dtt")
                nc.scalar.activation(dtt[:], evs[j][:], AF.Ln, bias=1.0)
                nc.sync.dma_start(dt_d[j], dtt[:])

        # ---------- phase D: selective scan + per-block out_proj/LN ----------
        with tc.tile_pool(name="pbc", bufs=1) as pBC, \
             tc.tile_pool(name="pd_s", bufs=2) as pS, \
             tc.tile_pool(name="pd_w", bufs=1) as pW, \
             tc.tile_pool(name="pyg", bufs=2) as pYG, \
             tc.tile_pool(name="pe", bufs=2) as pE, \
             tc.tile_pool(name="pe_r", bufs=1) as pER, \
             tc.tile_pool(name="pe_ps", bufs=2, space="PSUM") as pE_ps:
            for blk in range(NBLK):
                tsl = slice(blk * TB, (blk + 1) * TB)
                Bb = pBC.tile([128, DSTATE * TB], bf16, tag="Bb", name=f"Bb{blk}")
                Cb = pBC.tile([128, DSTATE * TB], bf16, tag="Cb", name=f"Cb{blk}")
                # quarter-split so the broadcast spreads over 4 DMA queues
                for q in range(4):
                    ssl = slice(q * 4, (q + 1) * 4)
                    fsl = slice(q * 4 * TB, (q + 1) * 4 * TB)
                    nc.gpsimd.dma_start(
                        Bb[:, fsl], bc_d[0, ssl, tsl].partition_broadcast(128))
                    nc.gpsimd.dma_start(
                        Cb[:, fsl], bc_d[1, ssl, tsl].partition_broadcast(128))

                ygs = [pYG.tile([128, TB], bf16, tag=f"yg{i}", name=f"yg{blk}_{i}")
                       for i in range(NCH)]
                for i in range(NCH):
                    dt_t = pS.tile([128, TB], bf16, tag="dtt")
                    nc.sync.dma_start(dt_t[:], dt_d[i, :, tsl])
                    xc_t = pS.tile([128, TB], bf16, tag="xctd")
                    nc.sync.dma_start(xc_t[:], xc_d[i, :, tsl])
                    zs_t = pS.tile([128, TB], bf16, tag="zstd")
                    nc.sync.dma_start(zs_t[:], zs_d[i, :, tsl])

                    # a8[n] = exp(A[:,n] * dt)  (16 ACT ops; double-buffered
                    # so ACT runs one iteration ahead of the DVE scan)
                    a8 = pW.tile([128, DSTATE * TB], bf16, tag="a8", bufs=2)
                    for n in range(DSTATE):
                        nc.scalar.activation(
                            a8[:, n * TB:(n + 1) * TB], dt_t[:], AF.Exp,
                            scale=at_all[:, i * DSTATE + n:i * DSTATE + n + 1])

                    dtx = pS.tile([128, TB], bf16, tag="dtx")
                    nc.vector.tensor_tensor(dtx[:], dt_t[:], xc_t[:], OP.mult)
                    u8 = pW.tile([128, DSTATE * TB], bf16, tag="u8")
                    nc.vector.tensor_tensor(
                        u8[:], dtx[:][:, None, :].broadcast_to([128, DSTATE, TB]),
                        Bb[:].rearrange("p (s t) -> p s t", s=DSTATE), OP.mult)

                    a8v = a8[:].rearrange("p (s t) -> p s t", s=DSTATE)
                    u8v = u8[:].rearrange("p (s t) -> p s t", s=DSTATE)
                    if blk > 0:
                        tmp = pS.tile([128, DSTATE], f32, tag="cytmp")
                        nc.vector.tensor_tensor(
                            tmp[:], a8v[:, :, 0:1].squeeze(), cys[i][:], OP.mult)
                        nc.vector.tensor_tensor(
                            u8v[:, :, 0:1].squeeze(),
                            u8v[:, :, 0:1].squeeze(), tmp[:], OP.add)
                    nc.vector.memset(a8v[:, :, 0:1], 0.0)

                    h8 = pW.tile([128, DSTATE * TB], bf16, tag="h8")
                    nc.vector.tensor_tensor_scan(h8[:], a8[:], u8[:], 0.0,
                                                 OP.mult, OP.add)
                    if blk < NBLK - 1:
                        nc.vector.tensor_copy(
                            cys[i][:],
                            h8[:].rearrange("p (s t) -> p s t",
                                            s=DSTATE)[:, :, TB - 1:TB].squeeze())

                    # y = sum_n C_n * h_n  (mult + pairwise tree, all bf16);
                    # u8 is dead after the scan, reuse its buffer for prod
                    prod = pW.tile([128, DSTATE * TB], bf16, tag="u8",
                                   name=f"prod{blk}_{i}")
                    nc.vector.tensor_tensor(prod[:], h8[:], Cb[:], OP.mult)
                    nc.vector.tensor_tensor(prod[:, 0:8 * TB], prod[:, 0:8 * TB],
                                            prod[:, 8 * TB:16 * TB], OP.add)
                    nc.vector.tensor_tensor(prod[:, 0:4 * TB], prod[:, 0:4 * TB],
                                            prod[:, 4 * TB:8 * TB], OP.add)
                    nc.vector.tensor_tensor(prod[:, 0:2 * TB], prod[:, 0:2 * TB],
                                            prod[:, 2 * TB:4 * TB], OP.add)
                    y1 = pS.tile([128, TB], bf16, tag="y1")
                    nc.vector.tensor_tensor(y1[:], prod[:, 0:TB],
                                            prod[:, TB:2 * TB], OP.add)
                    # y += D*xc ; gate with silu(z)
                    y2 = pS.tile([128, TB], bf16, tag="y2")
                    nc.vector.scalar_tensor_tensor(
                        y2[:], xc_t[:], dDc[:, i:i + 1], y1[:],
                        op0=OP.mult, op1=OP.add)
                    nc.vector.tensor_tensor(ygs[i][:], y2[:], zs_t[:], OP.mult)

                # ---- out_proj + residual + LayerNorm for this block ----
                rts = [pER.tile([128, DIM], bf16, tag=f"rt{t4}",
                                name=f"rt{blk}_{t4}") for t4 in range(TB // 128)]
                mv = pER.tile([128, 2 * (TB // 128)], f32, tag="mv",
                              name=f"mv{blk}")
                for t4 in range(TB // 128):
                    trow = blk * TB + t4 * 128
                    xres = pE.tile([128, DIM], bf16, tag="xres")
                    nc.sync.dma_start(xres[:], xbf_d[trow:trow + 128, :])
                    ps1 = pE_ps.tile([128, 512], f32, tag="op1")
                    ps2 = pE_ps.tile([128, 256], f32, tag="op2")
                    nc.tensor.matmul(ps1[:], identb[:], xres[:, 0:512],
                                     start=True, stop=False)
                    nc.tensor.matmul(ps2[:], identb[:], xres[:, 512:768],
                                     start=True, stop=False)
                    for i in range(NCH):
                        lhs = ygs[i][:, t4 * 128:(t4 + 1) * 128]
                        nc.tensor.matmul(ps1[:], lhs, woutb[i][:, 0:512],
                                         start=False, stop=(i == NCH - 1))
                        nc.tensor.matmul(ps2[:], lhs, woutb[i][:, 512:768],
                                         start=False, stop=(i == NCH - 1))
                    nc.scalar.copy(rts[t4][:, 0:512], ps1[:])
                    nc.scalar.copy(rts[t4][:, 512:768], ps2[:])
                    st6 = pE.tile([128, 12], f32, tag="st6")
                    nc.vector.bn_stats(st6[:, 0:6], rts[t4][:, 0:512])
                    nc.vector.bn_stats(st6[:, 6:12], rts[t4][:, 512:768])
                    nc.vector.bn_aggr(mv[:, 2 * t4:2 * t4 + 2],
                                      st6[:].rearrange("p (g s) -> p g s", g=2))

                # rstd = exp(-0.5*ln(var+eps)), batched over the block
                vare = pE.tile([128, TB // 128], f32, tag="vare")
                nc.vector.tensor_scalar(
                    vare[:], mv[:].rearrange("p (t s) -> p t s", s=2)[:, :, 1:2].squeeze(),
                    epsc[:], None, op0=OP.add)
                lnv = pE.tile([128, TB // 128], f32, tag="lnv")
                nc.scalar.activation(lnv[:], vare[:], AF.Ln)
                rstd = pE.tile([128, TB // 128], f32, tag="rstd", bufs=2)
                nc.scalar.activation(rstd[:], lnv[:], AF.Exp, scale=-0.5)

                for t4 in range(TB // 128):
                    trow = blk * TB + t4 * 128
                    t1 = pE.tile([128, DIM], bf16, tag="t1")
                    nc.vector.tensor_scalar(
                        t1[:], rts[t4][:], mv[:, 2 * t4:2 * t4 + 1],
                        rstd[:, t4:t4 + 1], op0=OP.subtract, op1=OP.mult)
                    t2 = pE.tile([128, DIM], bf16, tag="t2")
                    nc.vector.tensor_tensor(t2[:], t1[:], lngb[:], OP.mult)
                    of = pE.tile([128, DIM], f32, tag="of")
                    nc.vector.tensor_tensor(of[:], t2[:], lnbb[:], OP.add)
                    nc.sync.dma_start(out_d[trow:trow + 128, :], of[:])

    nc.compile()
    return nc


def _get_nc():
    if "nc" not in _CACHE:
        _CACHE["nc"] = _build()
    return _CACHE["nc"]


def kernel(**inputs):
    from concourse.bass_utils import run_bass_kernel_spmd

    nc = _get_nc()
    shared = {k: np.ascontiguousarray(np.asarray(inputs[k], np.float32))
              for k in ("sp_w1", "sp_b1", "sp_w2", "sp_b2", "in_proj_w", "conv_w",
                        "conv_b", "x_proj_w", "dt_proj_w", "dt_proj_b", "A_log",
                        "D", "out_proj_w", "ln_g", "ln_b")}
    x = np.asarray(inputs["x"], np.float32)
    sal = np.asarray(inputs["saliency_score"], np.float32)
    in_maps = []
    for c in range(B):
        m = dict(shared)
        m["x"] = np.ascontiguousarray(x[c])
        m["sal"] = np.ascontiguousarray(sal[c])
        in_maps.append(m)
    res = run_bass_kernel_spmd(nc, in_maps, core_ids=list(range(B)))
    out = np.stack([res.results[c]["out"] for c in range(B)], axis=0)
    return out
